# revision 1
# baseline (speedup 1.0000x reference)
"""Trainium2 Bass kernel for nn_Attention_867583394433 (sparse window attention).

Strategy (8 NeuronCores, pure data parallel over windows B_=256 -> 32/core):
  - Host precomputes the tiny position-MLP -> relative-position-bias table and
    folds it with the additive mask into a multiplicative table
    EM[mask, head] = exp(rpb + mask) (bf16), laid out to match the on-chip
    transposed-score layout.  Windows are assigned so each core only touches
    8 distinct masks (mask index = b % 64) and EM stays SBUF-resident.
  - Device computes, per window, in transposed score layout S^T[m, n]
    (key index m on partitions, query index n on free dim):
      qk^T channel-layout matmuls -> scores (K=32, row-tiled)
      -> exp on ScalarE -> P = exp(S^T) * EM on VectorE (bf16)
      -> PV and denominator (ones-matmul, col-tiled into matching partition
         rows) on TensorE -> fast reciprocal + fused normalize on VectorE
      -> output projection -> DMA out.
  - Biases are folded in by augmenting x^T / attnout^T with a ones row and the
    weights with a bias row; the q-scale is folded into w_q on the host.
"""

import os

import numpy as np

P16 = bool(int(os.environ.get("KERNEL_P16", "1")))  # 16-bit softmax path (fp16)

HEADS = 6
D = 32
C = 192
N = 256
B = 256
NMASK = 64
POS_DIM = 12
EPS = 1e-5
NCORES = 8
WPC = B // NCORES  # 32 windows per core
MPC = NMASK // NCORES  # 8 masks per core
REP = B // NMASK  # 4 windows sharing one mask
FREE = HEADS * 2 * N  # 3072: free layout (head, mtile, n)

_CACHE = {}


def _win_to_b(core, w):
    """Window order within a core: mask-major.  w = j*REP + k  ->  b."""
    j, k = divmod(w, REP)
    return NMASK * k + MPC * core + j


def _ln_np(x, g, b):
    m = x.mean(-1, keepdims=True)
    v = x.var(-1, keepdims=True)
    return (x - m) / np.sqrt(v + EPS) * g + b


def _pos_bias_host(H, W, pw0, pb0, g1, be1, w1, b1, g2, be2, w2, b2, g3, be3, w3, b3):
    """Replicates the reference position MLP + gather -> rpb [N, N, HEADS]."""
    H = int(H)
    W = int(W)
    ph = np.arange(1 - H, H)
    pw = np.arange(1 - W, W)
    biases = (
        np.stack(np.meshgrid(ph, pw, indexing="ij")).reshape(2, -1).T.astype(np.float32)
    )
    pos = biases @ pw0 + pb0
    pos = np.maximum(_ln_np(pos, g1, be1), 0.0) @ w1 + b1
    pos = np.maximum(_ln_np(pos, g2, be2), 0.0) @ w2 + b2
    pos = np.maximum(_ln_np(pos, g3, be3), 0.0) @ w3 + b3
    coords = np.stack(np.meshgrid(np.arange(H), np.arange(W), indexing="ij")).reshape(
        2, -1
    )
    rel = coords[:, :, None] - coords[:, None, :]
    rpi = (rel[0] + H - 1) * (2 * W - 1) + (rel[1] + W - 1)
    return pos[rpi]  # [N, N, HEADS] fp32


def _build_nc(repeat=1):
    import concourse.tile as tile
    from concourse import bacc, mybir

    FP = mybir.dt.float32
    BF = mybir.dt.float16 if P16 else mybir.dt.float32
    EXP = mybir.ActivationFunctionType.Exp
    MUL = mybir.AluOpType.mult

    nc = bacc.Bacc("TRN2", target_bir_lowering=False, debug=False)
    xt_d = nc.dram_tensor("xt", [WPC, 193, N], BF, kind="ExternalInput")
    em_d = nc.dram_tensor("em", [MPC, 128, FREE], BF, kind="ExternalInput")
    wqk_d = nc.dram_tensor("wqk", [193, 512], BF, kind="ExternalInput")
    wv_d = nc.dram_tensor("wv", [193, C], BF, kind="ExternalInput")
    wp_d = nc.dram_tensor("wp", [193, C], BF, kind="ExternalInput")
    y_d = nc.dram_tensor("y", [WPC, 128, 2, C], FP, kind="ExternalOutput")

    with tile.TileContext(nc) as tc:
        with (
            tc.tile_pool(name="const", bufs=1) as cpool,
            tc.tile_pool(name="win", bufs=int(os.environ.get("WBUFS", "2"))) as wpool,
            tc.tile_pool(name="big", bufs=int(os.environ.get("BBUFS", "3"))) as bpool,
            tc.tile_pool(name="ps_sc", bufs=2, space="PSUM") as ps_sc,
            tc.tile_pool(name="ps_m1", bufs=int(os.environ.get("M1BUFS", "2")), space="PSUM") as ps_m1,
            tc.tile_pool(name="ps_pv", bufs=1, space="PSUM") as ps_pv,
            tc.tile_pool(name="ps_dn", bufs=1, space="PSUM") as ps_dn,
        ):
            # ---- resident constants ----
            em_sb = cpool.tile([128, MPC, FREE], BF)
            em_loaded = set()
            wqk_sb = cpool.tile([128, 2, 512], BF)
            nc.sync.dma_start(wqk_sb[:, 0, :], wqk_d[0:128, :])
            nc.sync.dma_start(wqk_sb[0:65, 1, :], wqk_d[128:193, :])
            wv_sb = cpool.tile([128, 2, C], BF)
            nc.sync.dma_start(wv_sb[:, 0, :], wv_d[0:128, :])
            nc.sync.dma_start(wv_sb[0:65, 1, :], wv_d[128:193, :])
            wp_sb = cpool.tile([128, 2, C], BF)
            nc.sync.dma_start(wp_sb[:, 0, :], wp_d[0:128, :])
            nc.sync.dma_start(wp_sb[0:65, 1, :], wp_d[128:193, :])
            ones32 = cpool.tile([128, 32], BF)
            nc.gpsimd.memset(ones32[:], 1.0)

            # scores head -> (qk m-tile, partition row) maps
            q_loc = [(0, 32 * h) for h in range(4)] + [(2, 32 * (h - 4)) for h in (4, 5)]
            k_loc = [(1, 32 * h) for h in range(4)] + [(3, 32 * (h - 4)) for h in (4, 5)]

            def stage1a(w):
                """DMA x^T, qk^T matmuls + copy, v matmuls + copy, for window w."""
                j = w // REP
                if j not in em_loaded:
                    em_loaded.add(j)
                    nc.sync.dma_start(em_sb[:, j, :], em_d[j])
                xa = wpool.tile([128, 2, N], BF, tag="xa")
                nc.sync.dma_start(xa[:, 0, :], xt_d[w, 0:128, :])
                nc.sync.dma_start(xa[0:65, 1, :], xt_d[w, 128:193, :])

                if os.environ.get("QK_POOL", "sc") == "m1":
                    qkps = ps_m1.tile([128, 4, N], FP, tag="m1")
                else:
                    qkps = ps_sc.tile([128, 4, N], FP, tag="sc")
                for m in range(4):
                    nc.tensor.matmul(
                        qkps[:, m, :],
                        wqk_sb[:, 0, 128 * m : 128 * (m + 1)],
                        xa[:, 0, :],
                        start=True,
                        stop=False,
                    )
                    nc.tensor.matmul(
                        qkps[:, m, :],
                        wqk_sb[0:65, 1, 128 * m : 128 * (m + 1)],
                        xa[0:65, 1, :],
                        start=False,
                        stop=True,
                    )
                qkT = wpool.tile([128, 4, N], BF, tag="qkT")
                nc.vector.tensor_copy(qkT[:], qkps[:])

                vps = ps_m1.tile([128, 2, C], FP, tag="m1")
                for mt in range(2):
                    nc.tensor.matmul(
                        vps[:, mt, :],
                        xa[:, 0, 128 * mt : 128 * (mt + 1)],
                        wv_sb[:, 0, :],
                        start=True,
                        stop=False,
                    )
                    nc.tensor.matmul(
                        vps[:, mt, :],
                        xa[0:65, 1, 128 * mt : 128 * (mt + 1)],
                        wv_sb[0:65, 1, :],
                        start=False,
                        stop=True,
                    )
                vsb = wpool.tile([128, 2, C], BF, tag="vsb")
                nc.scalar.copy(vsb[:], vps[:])
                es = bpool.tile([128, FREE], BF, tag="es")
                return {"j": j, "qkT": qkT, "vsb": vsb, "es": es}

            def scores_phase(st, ph):
                """2 heads of S^T matmuls + one exp for this window."""
                qkT = st["qkT"]
                es = st["es"]
                scps = ps_sc.tile([128, 1024], FP, tag="sc")
                for hh in range(2):
                    h = 2 * ph + hh
                    qt, qr = q_loc[h]
                    kt, kr = k_loc[h]
                    for mt in range(2):
                        nc.tensor.matmul(
                            scps[:, 512 * hh + N * mt : 512 * hh + N * (mt + 1)],
                            qkT[kr : kr + 32, kt, 128 * mt : 128 * (mt + 1)],
                            qkT[qr : qr + 32, qt, :],
                            start=True,
                            stop=True,
                            tile_position=(kr, 0),
                        )
                nc.scalar.activation(es[:, 1024 * ph : 1024 * (ph + 1)], scps[:], EXP)

            def p_mult(st):
                """P(w) = exp(S^T) * EM -- emitted at the START of iter w+1."""
                p_t = bpool.tile([128, FREE], BF, tag="P")
                nc.vector.tensor_tensor(p_t[:], st["es"][:], em_sb[:, st["j"], :], MUL)
                st["p"] = p_t
                st["pvps"] = ps_pv.tile([128, 512], FP, tag="pv", name="pvps")
                st["dnps"] = ps_dn.tile([128, 512], FP, tag="dn", name="dnps")

            def pv_den_chunk(st, ph):
                """2 heads of PV + den matmuls for the previous window."""
                p_t = st["p"]
                vsb = st["vsb"]
                for h in (2 * ph, 2 * ph + 1):
                    cb = (32 * h) % 128
                    fo = 0 if h < 4 else N
                    for mt in range(2):
                        rhs = p_t[:, 512 * h + N * mt : 512 * h + N * (mt + 1)]
                        nc.tensor.matmul(
                            st["pvps"][cb : cb + 32, fo : fo + N],
                            vsb[:, mt, 32 * h : 32 * (h + 1)],
                            rhs,
                            start=(mt == 0),
                            stop=(mt == 1),
                            tile_position=(0, cb),
                        )
                        nc.tensor.matmul(
                            st["dnps"][cb : cb + 32, fo : fo + N],
                            ones32[:],
                            rhs,
                            start=(mt == 0),
                            stop=(mt == 1),
                            tile_position=(0, cb),
                        )

            def stage2b(w, st):
                """Normalize, project, and store window w (the previous one)."""
                pvps = st["pvps"]
                dnps = st["dnps"]
                ivd = wpool.tile([128, 512], FP, tag="ivd")
                nc.vector.reciprocal_approx_fast(ivd[:], dnps[:])
                aoT = wpool.tile([128, 2, N], BF, tag="aoT")
                nc.vector.tensor_tensor(
                    aoT[:].rearrange("p t n -> p (t n)"), pvps[:], ivd[:], MUL
                )
                nc.gpsimd.memset(aoT[64:65, 1, :], 1.0)
                yps = ps_m1.tile([128, 2, C], FP, tag="m1")
                for mt in range(2):
                    nc.tensor.matmul(
                        yps[:, mt, :],
                        aoT[:, 0, 128 * mt : 128 * (mt + 1)],
                        wp_sb[:, 0, :],
                        start=True,
                        stop=False,
                    )
                    nc.tensor.matmul(
                        yps[:, mt, :],
                        aoT[0:65, 1, 128 * mt : 128 * (mt + 1)],
                        wp_sb[0:65, 1, :],
                        start=False,
                        stop=True,
                    )
                ysb = wpool.tile([128, 2, C], FP, tag="ysb")
                nc.vector.tensor_copy(ysb[:], yps[:])
                nc.sync.dma_start(y_d[w], ysb[:])

            prev = None
            prev_w = None
            for rep in range(repeat):
                for it in range(WPC):
                    if prev is not None:
                        p_mult(prev)
                    cur = stage1a(it)
                    ivmode = os.environ.get("ILV", "0")
                    for ph in range(3):
                        if prev is not None and ivmode == "1":
                            pv_den_chunk(prev, ph)
                        scores_phase(cur, ph)
                    if prev is not None:
                        if ivmode != "1":
                            for ph in range(3):
                                pv_den_chunk(prev, ph)
                        stage2b(prev_w, prev)
                    prev, prev_w = cur, it
            p_mult(prev)
            for ph in range(3):
                pv_den_chunk(prev, ph)
            stage2b(prev_w, prev)

    nc.compile()
    return nc


def _prep_inputs(inputs):
    x = np.asarray(inputs["x"], np.float32)
    mask = np.asarray(inputs["mask"], np.float32)
    w_qkv = np.asarray(inputs["w_qkv"], np.float32)
    b_qkv = np.asarray(inputs["b_qkv"], np.float32)
    w_proj = np.asarray(inputs["w_proj"], np.float32)
    b_proj = np.asarray(inputs["b_proj"], np.float32)
    H, W = int(inputs["H"]), int(inputs["W"])

    scale = float(D) ** -0.5
    rpb = _pos_bias_host(
        H,
        W,
        *[
            np.asarray(inputs[k], np.float32)
            for k in (
                "pw0",
                "pb0",
                "g1",
                "be1",
                "w1",
                "b1",
                "g2",
                "be2",
                "w2",
                "b2",
                "g3",
                "be3",
                "w3",
                "b3",
            )
        ],
    )

    # EM[mb, p, h*512 + mt*256 + n] = exp(mask[mb, n, m] + rpb[n, m, h]), m = mt*128+p
    bias = mask.transpose(0, 2, 1)[:, None] + rpb.transpose(2, 1, 0)[None]
    em = np.exp(bias)  # [64, 6, 256(m), 256(n)]
    em = em.reshape(NMASK, HEADS, 2, 128, N).transpose(0, 3, 1, 2, 4)
    em = np.ascontiguousarray(em.reshape(NMASK, 128, FREE)).astype(np.float16 if P16 else np.float32)

    # packed/augmented weights
    wq = np.vstack([w_qkv[:, 0:C] * scale, (b_qkv[0:C] * scale)[None]])  # [193, 192]
    wk = np.vstack([w_qkv[:, C : 2 * C], b_qkv[C : 2 * C][None]])
    mmdt = np.float16 if P16 else np.float32
    wqk = np.zeros((193, 512), np.float32)
    wqk[:, 0:128] = wq[:, 0:128]
    wqk[:, 128:256] = wk[:, 0:128]
    wqk[:, 256:320] = wq[:, 128:192]
    wqk[:, 384:448] = wk[:, 128:192]
    wqk = wqk.astype(mmdt)
    wv = np.ascontiguousarray(np.vstack([w_qkv[:, 2 * C :], b_qkv[2 * C :][None]])).astype(mmdt)
    wp = np.ascontiguousarray(np.vstack([w_proj, b_proj[None]])).astype(mmdt)

    # per-core x^T with ones row
    xt_aug = np.empty((B, 193, N), mmdt)
    xt_aug[:, 0:C, :] = x.transpose(0, 2, 1)
    xt_aug[:, C, :] = 1.0

    in_maps = []
    for core in range(NCORES):
        bs = [_win_to_b(core, w) for w in range(WPC)]
        in_maps.append(
            {
                "xt": np.ascontiguousarray(xt_aug[bs]),
                "em": np.ascontiguousarray(em[MPC * core : MPC * (core + 1)]),
                "wqk": wqk,
                "wv": wv,
                "wp": wp,
            }
        )
    return in_maps


def _assemble(results):
    out = np.empty((B, N, C), np.float32)
    for core in range(NCORES):
        y = results[core]["y"]  # [WPC, 128, 2, C]
        for w in range(WPC):
            b = _win_to_b(core, w)
            out[b] = y[w].transpose(1, 0, 2).reshape(N, C)
    return out


def run(inputs, trace=False):
    from concourse.bass_utils import run_bass_kernel_spmd

    if "nc" not in _CACHE:
        _CACHE["nc"] = _build_nc()
    in_maps = _prep_inputs(inputs)
    res = run_bass_kernel_spmd(
        _CACHE["nc"],
        in_maps,
        core_ids=list(range(NCORES)),
        trace=trace,
        trace_cores=[0] if trace else None,
    )
    return _assemble(res.results), res


def get_nc():
    if "nc" not in _CACHE:
        _CACHE["nc"] = _build_nc()
    return _CACHE["nc"]


def kernel(**inputs):
    out, _ = run(inputs, trace=bool(int(os.environ.get("KERNEL_TRACE", "0"))))
    return out



# revision 5
# speedup vs baseline: 1.0908x; 1.0908x over previous
"""Trainium2 Bass kernel for nn_Attention_867583394433 (sparse window attention).

Strategy (8 NeuronCores, pure data parallel over windows B_=256 -> 32/core):
  - Host precomputes the tiny position-MLP -> relative-position-bias table and
    folds it with the additive mask into a multiplicative table
    EM[mask, head] = exp(rpb + mask) (fp16), laid out to match the on-chip
    transposed-score layout.  Windows are assigned so each core only touches
    8 distinct masks (mask index = b % 64) and EM stays SBUF-resident.
  - Device computes, per window, in transposed score layout S^T[m, n]
    (key index m on partitions, query index n on free dim):
      qk^T channel-layout matmuls -> scores (K=32, row-tiled)
      -> exp on ScalarE -> P = exp(S^T) * EM elementwise, split between
      VectorE and GpSimd (both only touch SBUF) to balance engines
      -> PV and denominator (ones-matmul, col-tiled into matching partition
         rows, packed [96, 2, 256]) on TensorE -> fast reciprocal + fused
         normalize on VectorE -> output projection -> fp16 DMA out.
  - The d^-0.5 attention scale is folded into wq on the host; the (zero)
    qkv/proj biases are folded into a host-side constant correction.
"""

import os

import numpy as np

HEADS = 6
D = 32
C = 192
N = 256
B = 256
NMASK = 64
POS_DIM = 12
EPS = 1e-5
NCORES = 8
WPC = B // NCORES  # 32 windows per core
MPC = NMASK // NCORES  # 8 masks per core
REP = B // NMASK  # 4 windows sharing one mask

S_AO = 16.0  # aoT = S_AO * attnout_true (via the ones value)
ONES_VAL = 1.0 / S_AO
Y_DESCALE = 1.0 / S_AO

# head split of the P = exp(S)*EM multiply: first PSPLIT heads on DVE,
# the rest on GpSimd.
PSPLIT = int(os.environ.get("PSPLIT", "4"))

_CACHE = {}


def _win_to_b(core, w):
    """Window order within a core: mask-major.  w = j*REP + k  ->  b."""
    j, k = divmod(w, REP)
    return NMASK * k + MPC * core + j


def _ln_np(x, g, b):
    m = x.mean(-1, keepdims=True)
    v = x.var(-1, keepdims=True)
    return (x - m) / np.sqrt(v + EPS) * g + b


def _pos_bias_host(H, W, pw0, pb0, g1, be1, w1, b1, g2, be2, w2, b2, g3, be3, w3, b3):
    """Replicates the reference position MLP + gather -> rpb [N, N, HEADS]."""
    H = int(H)
    W = int(W)
    ph = np.arange(1 - H, H)
    pw = np.arange(1 - W, W)
    biases = (
        np.stack(np.meshgrid(ph, pw, indexing="ij")).reshape(2, -1).T.astype(np.float32)
    )
    pos = biases @ pw0 + pb0
    pos = np.maximum(_ln_np(pos, g1, be1), 0.0) @ w1 + b1
    pos = np.maximum(_ln_np(pos, g2, be2), 0.0) @ w2 + b2
    pos = np.maximum(_ln_np(pos, g3, be3), 0.0) @ w3 + b3
    coords = np.stack(np.meshgrid(np.arange(H), np.arange(W), indexing="ij")).reshape(
        2, -1
    )
    rel = coords[:, :, None] - coords[:, None, :]
    rpi = (rel[0] + H - 1) * (2 * W - 1) + (rel[1] + W - 1)
    return pos[rpi]  # [N, N, HEADS] fp32


def _build_nc():
    import concourse.tile as tile
    from concourse import bacc, mybir

    FP = mybir.dt.float32
    F16 = mybir.dt.float16
    EXP = mybir.ActivationFunctionType.Exp
    COPY = mybir.ActivationFunctionType.Copy
    MUL = mybir.AluOpType.mult

    nc = bacc.Bacc("TRN2", target_bir_lowering=False, debug=False)
    # x^T feature chunks: [w, chunk(2), 128, 256]; chunk 1 holds features
    # 128:192 in rows 0:64 (rows 64:128 are zero padding).
    xt_d = nc.dram_tensor("xt", [WPC, 2, 128, N], F16, kind="ExternalInput")
    em_d = nc.dram_tensor("em", [MPC, 128, HEADS, 2, N], F16, kind="ExternalInput")
    wqk_d = nc.dram_tensor("wqk", [2, 128, 512], F16, kind="ExternalInput")
    wv_d = nc.dram_tensor("wv", [2, 128, C], F16, kind="ExternalInput")
    wp_d = nc.dram_tensor("wp", [96, 2, C], F16, kind="ExternalInput")
    y_d = nc.dram_tensor("y", [WPC, 128, 2, C], F16, kind="ExternalOutput")

    with tile.TileContext(nc) as tc:
        with (
            tc.tile_pool(name="const", bufs=1) as cpool,
            tc.tile_pool(name="win", bufs=int(os.environ.get("WBUFS", "2"))) as wpool,
            tc.tile_pool(name="big", bufs=int(os.environ.get("BBUFS", "3"))) as bpool,
            tc.tile_pool(name="pbig", bufs=int(os.environ.get("PBUFS", "2"))) as ppool,
            tc.tile_pool(name="ps_sc", bufs=2, space="PSUM") as ps_sc,
            tc.tile_pool(name="ps_m1", bufs=2, space="PSUM") as ps_m1,
            tc.tile_pool(name="ps_pv", bufs=1, space="PSUM") as ps_pv,
            tc.tile_pool(name="ps_dn", bufs=1, space="PSUM") as ps_dn,
        ):
            # ---- resident constants ----
            em_sb = cpool.tile([128, MPC, HEADS, 2, N], F16)
            em_loaded = set()
            wqk_sb = cpool.tile([128, 2, 512], F16)
            nc.sync.dma_start(wqk_sb[:, 0], wqk_d[0])
            nc.sync.dma_start(wqk_sb[:, 1], wqk_d[1])
            wv_sb = cpool.tile([128, 2, C], F16)
            nc.sync.dma_start(wv_sb[:, 0], wv_d[0])
            nc.sync.dma_start(wv_sb[:, 1], wv_d[1])
            wp_sb = cpool.tile([96, 2, C], F16)
            nc.sync.dma_start(wp_sb[:], wp_d[:])
            ones16 = cpool.tile([128, D], F16)
            nc.gpsimd.memset(ones16[:], ONES_VAL)

            # scores head -> (qk m-tile, partition row) maps
            q_loc = [(0, 32 * h) for h in range(4)] + [(2, 32 * (h - 4)) for h in (4, 5)]
            k_loc = [(1, 32 * h) for h in range(4)] + [(3, 32 * (h - 4)) for h in (4, 5)]

            def stage1a(w):
                """DMA x^T, qk matmuls + copy, v matmuls + copy, for window w."""
                j = w // REP
                if j not in em_loaded:
                    em_loaded.add(j)
                    nc.sync.dma_start(em_sb[:, j], em_d[j])
                xa = wpool.tile([128, 2, N], F16, tag="xa")
                nc.sync.dma_start(xa[:, 0, :], xt_d[w, 0])
                nc.sync.dma_start(xa[0:64, 1, :], xt_d[w, 1, 0:64])

                qkps = ps_sc.tile([128, 4, N], FP, tag="sc")
                for t in range(4):
                    nc.tensor.matmul(
                        qkps[:, t, :],
                        wqk_sb[:, 0, 128 * t : 128 * (t + 1)],
                        xa[:, 0, :],
                        start=True,
                        stop=False,
                    )
                    nc.tensor.matmul(
                        qkps[:, t, :],
                        wqk_sb[0:64, 1, 128 * t : 128 * (t + 1)],
                        xa[0:64, 1, :],
                        start=False,
                        stop=True,
                    )
                qkT = wpool.tile([128, 4, N], F16, tag="qkT")
                nc.vector.tensor_copy(qkT[:], qkps[:])

                vps = ps_m1.tile([128, 2, C], FP, tag="m1")
                for mt in range(2):
                    nc.tensor.matmul(
                        vps[:, mt, :],
                        xa[:, 0, 128 * mt : 128 * (mt + 1)],
                        wv_sb[:, 0, :],
                        start=True,
                        stop=False,
                    )
                    nc.tensor.matmul(
                        vps[:, mt, :],
                        xa[0:64, 1, 128 * mt : 128 * (mt + 1)],
                        wv_sb[0:64, 1, :],
                        start=False,
                        stop=True,
                    )
                vsb = wpool.tile([128, 2, C], F16, tag="vsb")
                nc.scalar.copy(vsb[:], vps[:])
                es = bpool.tile([128, HEADS, 2, N], F16, tag="es")
                return {"j": j, "qkT": qkT, "vsb": vsb, "es": es}

            def scores_phase(st, ph):
                """2 heads of S^T matmuls + one exp for this window."""
                qkT = st["qkT"]
                es = st["es"]
                scps = ps_sc.tile([128, 2, 2, N], FP, tag="sc")
                for hh in range(2):
                    h = 2 * ph + hh
                    qt, qr = q_loc[h]
                    kt, kr = k_loc[h]
                    for mt in range(2):
                        nc.tensor.matmul(
                            scps[:, hh, mt, :],
                            qkT[kr : kr + 32, kt, 128 * mt : 128 * (mt + 1)],
                            qkT[qr : qr + 32, qt, :],
                            start=True,
                            stop=True,
                            tile_position=(kr, 0),
                        )
                nc.scalar.activation(es[:, 2 * ph : 2 * ph + 2], scps[:], EXP)

            def p_mult(st):
                """P(w) = exp(S^T) * EM -- emitted at the START of iter w+1.

                Split along heads: first PSPLIT on VectorE, rest on GpSimd.
                """
                p_t = ppool.tile([128, HEADS, 2, N], F16, tag="P")
                j = st["j"]
                es = st["es"]
                if PSPLIT > 0:
                    nc.vector.tensor_tensor(
                        p_t[:, 0:PSPLIT], es[:, 0:PSPLIT], em_sb[:, j, 0:PSPLIT], MUL
                    )
                if PSPLIT < HEADS:
                    nc.gpsimd.tensor_tensor(
                        p_t[:, PSPLIT:], es[:, PSPLIT:], em_sb[:, j, PSPLIT:], MUL
                    )
                st["p"] = p_t
                st["pvps"] = ps_pv.tile([96, 2, N], FP, tag="pv", name="pvps")
                st["dnps"] = ps_dn.tile([96, 2, N], FP, tag="dn", name="dnps")

            def pv_den_chunk(st, ph):
                """2 heads of PV + den matmuls for the previous window."""
                p_t = st["p"]
                vsb = st["vsb"]
                for hh in range(2):
                    h = 2 * ph + hh
                    rb = 32 * (h % 3)
                    fo = h // 3
                    for mt in range(2):
                        rhs = p_t[:, h, mt, :]
                        nc.tensor.matmul(
                            st["pvps"][rb : rb + 32, fo, :],
                            vsb[:, mt, D * h : D * (h + 1)],
                            rhs,
                            start=(mt == 0),
                            stop=(mt == 1),
                            tile_position=(0, rb),
                        )
                        nc.tensor.matmul(
                            st["dnps"][rb : rb + 32, fo, :],
                            ones16[:],
                            rhs,
                            start=(mt == 0),
                            stop=(mt == 1),
                            tile_position=(0, rb),
                        )

            def stage2b(w, st):
                """Normalize, project, and store window w (the previous one)."""
                pvps = st["pvps"]
                dnps = st["dnps"]
                ivd = wpool.tile([96, 2, N], FP, tag="ivd")
                nc.vector.reciprocal_approx_fast(ivd[:], dnps[:])
                aoT = wpool.tile([96, 2, N], F16, tag="aoT")
                nc.vector.tensor_tensor(aoT[:], pvps[:], ivd[:], MUL)
                yps = ps_m1.tile([128, 2, C], FP, tag="m1")
                for mt in range(2):
                    for j in range(2):
                        nc.tensor.matmul(
                            yps[:, mt, :],
                            aoT[:, j, 128 * mt : 128 * (mt + 1)],
                            wp_sb[:, j, :],
                            start=(j == 0),
                            stop=(j == 1),
                        )
                ysb = wpool.tile([128, 2, C], F16, tag="ysb")
                nc.vector.tensor_scalar_mul(ysb[:], yps[:], Y_DESCALE)
                nc.sync.dma_start(y_d[w], ysb[:])

            prev = None
            prev_w = None
            for it in range(WPC):
                if prev is not None:
                    p_mult(prev)
                cur = stage1a(it)
                for ph in range(3):
                    if prev is not None:
                        pv_den_chunk(prev, ph)
                    scores_phase(cur, ph)
                if prev is not None:
                    stage2b(prev_w, prev)
                prev, prev_w = cur, it
            p_mult(prev)
            for ph in range(3):
                pv_den_chunk(prev, ph)
            stage2b(prev_w, prev)

    nc.compile()
    return nc


def _prep_inputs(inputs):
    x = np.asarray(inputs["x"], np.float32)
    mask = np.asarray(inputs["mask"], np.float32)
    w_qkv = np.asarray(inputs["w_qkv"], np.float32)
    b_qkv = np.asarray(inputs["b_qkv"], np.float32)
    w_proj = np.asarray(inputs["w_proj"], np.float32)
    b_proj = np.asarray(inputs["b_proj"], np.float32)
    H, W = int(inputs["H"]), int(inputs["W"])

    rpb = _pos_bias_host(
        H,
        W,
        *[
            np.asarray(inputs[k], np.float32)
            for k in (
                "pw0", "pb0", "g1", "be1", "w1", "b1",
                "g2", "be2", "w2", "b2", "g3", "be3", "w3", "b3",
            )
        ],
    )

    # EM[mb, h, m, n] = exp(mask[mb, n, m] + rpb[n, m, h]), device layout
    # [mb][p, h, mt, n] with m = 128*mt + p.
    bias = mask.transpose(0, 2, 1)[:, None] + rpb.transpose(2, 1, 0)[None]
    em = np.exp(bias).reshape(NMASK, HEADS, 2, 128, N).transpose(0, 3, 1, 2, 4)
    em = np.ascontiguousarray(em).astype(np.float16)  # [64, 128, 6, 2, 256]

    scale = float(D) ** -0.5
    wq = w_qkv[:, 0:C] * scale
    wk = w_qkv[:, C : 2 * C]
    wqk = np.zeros((C, 512), np.float32)
    wqk[:, 0:128] = wq[:, 0:128]
    wqk[:, 128:256] = wk[:, 0:128]
    wqk[:, 256:320] = wq[:, 128:C]
    wqk[:, 384:448] = wk[:, 128:C]
    wqk16 = np.zeros((2, 128, 512), np.float16)
    wqk16[0] = wqk[0:128]
    wqk16[1, 0:64] = wqk[128:C]
    wv16 = np.zeros((2, 128, C), np.float16)
    wv16[0] = w_qkv[0:128, 2 * C :]
    wv16[1, 0:64] = w_qkv[128:C, 2 * C :]
    wp16 = np.ascontiguousarray(
        w_proj.reshape(2, 96, C).transpose(1, 0, 2)
    ).astype(np.float16)

    # per-core x^T chunks [w, 2, 128, 256]
    xt16 = np.zeros((B, 2, 128, N), np.float16)
    xtr = x.transpose(0, 2, 1)  # [B, C, N]
    xt16[:, 0] = xtr[:, 0:128]
    xt16[:, 1, 0:64] = xtr[:, 128:C]

    in_maps = []
    for core in range(NCORES):
        bs = [_win_to_b(core, w) for w in range(WPC)]
        in_maps.append(
            {
                "xt": np.ascontiguousarray(xt16[bs]),
                "em": np.ascontiguousarray(em[MPC * core : MPC * (core + 1)]),
                "wqk": wqk16,
                "wv": wv16,
                "wp": wp16,
            }
        )
    # host-side constant correction for (generally zero) v/proj biases.
    ycorr = None
    if np.any(b_qkv != 0.0) or np.any(b_proj != 0.0):
        bv = b_qkv[2 * C :]
        ycorr = (bv @ w_proj + b_proj).astype(np.float32)
        if np.any(b_qkv[: 2 * C] != 0.0):
            raise NotImplementedError("nonzero q/k bias not supported by fast path")
    return in_maps, ycorr


def _assemble(results, ycorr):
    out = np.empty((B, N, C), np.float32)
    for core in range(NCORES):
        y = results[core]["y"]  # [WPC, 128, 2, C] fp16
        for w in range(WPC):
            b = _win_to_b(core, w)
            out[b] = y[w].transpose(1, 0, 2).reshape(N, C).astype(np.float32)
    if ycorr is not None:
        out += ycorr
    return out


def run(inputs, trace=False):
    from concourse.bass_utils import run_bass_kernel_spmd

    if "nc" not in _CACHE:
        _CACHE["nc"] = _build_nc()
    in_maps, ycorr = _prep_inputs(inputs)
    res = run_bass_kernel_spmd(
        _CACHE["nc"],
        in_maps,
        core_ids=list(range(NCORES)),
        trace=trace,
        trace_cores=[0] if trace else None,
    )
    return _assemble(res.results, ycorr), res


def get_nc():
    if "nc" not in _CACHE:
        _CACHE["nc"] = _build_nc()
    return _CACHE["nc"]


def kernel(**inputs):
    out, _ = run(inputs, trace=bool(int(os.environ.get("KERNEL_TRACE", "0"))))
    return out


# revision 9
# speedup vs baseline: 1.1613x; 1.0646x over previous
"""Trainium2 Bass kernel for nn_Attention_867583394433 (sparse window attention).

Strategy (8 NeuronCores, pure data parallel over windows B_=256 -> 32/core):
  - Host precomputes the tiny position-MLP -> relative-position-bias table and
    folds it with the additive mask into a multiplicative table
    EM[mask, head] = exp(rpb + mask) (fp16), laid out to match the on-chip
    transposed-score layout.  Windows are assigned so each core only touches
    8 distinct masks (mask index = b % 64) and EM stays SBUF-resident.
  - Device computes, per window, in transposed score layout S^T[m, n]
    (key index m on partitions, query index n on free dim):
      qk^T channel-layout matmuls -> scores (K=32, row-tiled)
      -> exp on ScalarE -> P = exp(S^T) * EM elementwise, split between
      VectorE and GpSimd (both only touch SBUF) to balance engines
      -> PV and denominator (ones-matmul, col-tiled into matching partition
         rows, packed [96, 2, 256]) on TensorE -> fast reciprocal + fused
         normalize on VectorE -> output projection -> fp16 DMA out.
  - The d^-0.5 attention scale is folded into wq on the host; the (zero)
    qkv/proj biases are folded into a host-side constant correction.
"""

import os

import numpy as np

HEADS = 6
D = 32
C = 192
N = 256
B = 256
NMASK = 64
POS_DIM = 12
EPS = 1e-5
NCORES = 8
WPC = B // NCORES  # 32 windows per core
MPC = NMASK // NCORES  # 8 masks per core
REP = B // NMASK  # 4 windows sharing one mask

S_AO = 16.0  # aoT = S_AO * attnout_true (via the ones value)
ONES_VAL = 1.0 / S_AO
Y_DESCALE = 1.0 / S_AO

# head split of the P = exp(S)*EM multiply: first PSPLIT heads on DVE,
# the rest on GpSimd.
PSPLIT = int(os.environ.get("PSPLIT", "4"))

_CACHE = {}


def _win_to_b(core, w):
    """Window order within a core: mask-major.  w = j*REP + k  ->  b."""
    j, k = divmod(w, REP)
    return NMASK * k + MPC * core + j


def _ln_np(x, g, b):
    m = x.mean(-1, keepdims=True)
    v = x.var(-1, keepdims=True)
    return (x - m) / np.sqrt(v + EPS) * g + b


def _pos_bias_host(H, W, pw0, pb0, g1, be1, w1, b1, g2, be2, w2, b2, g3, be3, w3, b3):
    """Replicates the reference position MLP + gather -> rpb [N, N, HEADS]."""
    H = int(H)
    W = int(W)
    ph = np.arange(1 - H, H)
    pw = np.arange(1 - W, W)
    biases = (
        np.stack(np.meshgrid(ph, pw, indexing="ij")).reshape(2, -1).T.astype(np.float32)
    )
    pos = biases @ pw0 + pb0
    pos = np.maximum(_ln_np(pos, g1, be1), 0.0) @ w1 + b1
    pos = np.maximum(_ln_np(pos, g2, be2), 0.0) @ w2 + b2
    pos = np.maximum(_ln_np(pos, g3, be3), 0.0) @ w3 + b3
    coords = np.stack(np.meshgrid(np.arange(H), np.arange(W), indexing="ij")).reshape(
        2, -1
    )
    rel = coords[:, :, None] - coords[:, None, :]
    rpi = (rel[0] + H - 1) * (2 * W - 1) + (rel[1] + W - 1)
    return pos[rpi]  # [N, N, HEADS] fp32


def _build_nc():
    import concourse.tile as tile
    from concourse import bacc, mybir

    FP = mybir.dt.float32
    F16 = mybir.dt.float16
    EXP = mybir.ActivationFunctionType.Exp
    COPY = mybir.ActivationFunctionType.Copy
    MUL = mybir.AluOpType.mult

    nc = bacc.Bacc("TRN2", target_bir_lowering=False, debug=False)
    # x^T feature chunks: [w, chunk(2), 128, 256]; chunk 1 holds features
    # 128:192 in rows 0:64 (rows 64:128 are zero padding).
    xt_d = nc.dram_tensor("xt", [WPC, 2, 128, N], F16, kind="ExternalInput")
    em_d = nc.dram_tensor("em", [MPC, 128, HEADS, 2, N], F16, kind="ExternalInput")
    wqk_d = nc.dram_tensor("wqk", [2, 128, 512], F16, kind="ExternalInput")
    wv_d = nc.dram_tensor("wv", [2, 128, C], F16, kind="ExternalInput")
    wp_d = nc.dram_tensor("wp", [96, 2, C], F16, kind="ExternalInput")
    y_d = nc.dram_tensor("y", [WPC, 128, 2, C], F16, kind="ExternalOutput")

    with tile.TileContext(nc) as tc:
        with (
            tc.tile_pool(name="const", bufs=1) as cpool,
            tc.tile_pool(name="win", bufs=int(os.environ.get("WBUFS", "2"))) as wpool,
            tc.tile_pool(name="big", bufs=int(os.environ.get("BBUFS", "3"))) as bpool,
            tc.tile_pool(name="pbig", bufs=int(os.environ.get("PBUFS", "2"))) as ppool,
            tc.tile_pool(name="ps_sc", bufs=2, space="PSUM") as ps_sc,
            tc.tile_pool(name="ps_m1", bufs=2, space="PSUM") as ps_m1,
            tc.tile_pool(name="ps_pv", bufs=1, space="PSUM") as ps_pv,
            tc.tile_pool(name="ps_dn", bufs=1, space="PSUM") as ps_dn,
        ):
            # ---- resident constants ----
            em_sb = cpool.tile([128, MPC, HEADS, 2, N], F16)
            em_loaded = set()
            wqk_sb = cpool.tile([128, 2, 512], F16)
            nc.sync.dma_start(wqk_sb[:, 0], wqk_d[0])
            nc.sync.dma_start(wqk_sb[:, 1], wqk_d[1])
            wv_sb = cpool.tile([128, 2, C], F16)
            nc.sync.dma_start(wv_sb[:, 0], wv_d[0])
            nc.sync.dma_start(wv_sb[:, 1], wv_d[1])
            wp_sb = cpool.tile([96, 2, C], F16)
            nc.sync.dma_start(wp_sb[:], wp_d[:])
            ones16 = cpool.tile([128, D], F16)
            nc.gpsimd.memset(ones16[:], ONES_VAL)

            # scores head -> (qk m-tile, partition row) maps
            q_loc = [(0, 32 * h) for h in range(4)] + [(2, 32 * (h - 4)) for h in (4, 5)]
            k_loc = [(1, 32 * h) for h in range(4)] + [(3, 32 * (h - 4)) for h in (4, 5)]

            def stage1a(w):
                """DMA x^T, qk matmuls + copy, v matmuls + copy, for window w."""
                j = w // REP
                xa = wpool.tile([128, 2, N], F16, tag="xa")
                nc.sync.dma_start(xa[:, 0, :], xt_d[w, 0])
                nc.sync.dma_start(xa[0:64, 1, :], xt_d[w, 1, 0:64])
                if j not in em_loaded:
                    em_loaded.add(j)
                    nc.sync.dma_start(em_sb[:, j], em_d[j])

                qkps = ps_sc.tile([128, 4, N], FP, tag="sc")
                for t in range(4):
                    nc.tensor.matmul(
                        qkps[:, t, :],
                        wqk_sb[:, 0, 128 * t : 128 * (t + 1)],
                        xa[:, 0, :],
                        start=True,
                        stop=False,
                    )
                    nc.tensor.matmul(
                        qkps[:, t, :],
                        wqk_sb[0:64, 1, 128 * t : 128 * (t + 1)],
                        xa[0:64, 1, :],
                        start=False,
                        stop=True,
                    )
                qkT = wpool.tile([128, 4, N], F16, tag="qkT")
                nc.vector.tensor_copy(qkT[:, 0:2], qkps[:, 0:2])
                nc.scalar.copy(qkT[:, 2:4], qkps[:, 2:4])

                vps = ps_m1.tile([128, 2, C], FP, tag="m1")
                for mt in range(2):
                    nc.tensor.matmul(
                        vps[:, mt, :],
                        xa[:, 0, 128 * mt : 128 * (mt + 1)],
                        wv_sb[:, 0, :],
                        start=True,
                        stop=False,
                    )
                    nc.tensor.matmul(
                        vps[:, mt, :],
                        xa[0:64, 1, 128 * mt : 128 * (mt + 1)],
                        wv_sb[0:64, 1, :],
                        start=False,
                        stop=True,
                    )
                vsb = wpool.tile([128, 2, C], F16, tag="vsb")
                nc.scalar.copy(vsb[:], vps[:])
                es = bpool.tile([128, HEADS, 2, N], F16, tag="es")
                return {"j": j, "qkT": qkT, "vsb": vsb, "es": es}

            def scores_phase(st, ph):
                """2 heads of S^T matmuls + one exp for this window."""
                qkT = st["qkT"]
                es = st["es"]
                scps = ps_sc.tile([128, 2, 2, N], FP, tag="sc")
                for hh in range(2):
                    h = 2 * ph + hh
                    qt, qr = q_loc[h]
                    kt, kr = k_loc[h]
                    for mt in range(2):
                        nc.tensor.matmul(
                            scps[:, hh, mt, :],
                            qkT[kr : kr + 32, kt, 128 * mt : 128 * (mt + 1)],
                            qkT[qr : qr + 32, qt, :],
                            start=True,
                            stop=True,
                            tile_position=(kr, 0),
                        )
                nc.scalar.activation(es[:, 2 * ph : 2 * ph + 2], scps[:], EXP)

            def p_alloc(st):
                st["p"] = ppool.tile([128, HEADS, 2, N], F16, tag="P", name="p_t")

            def p_mult_chunk(st, ph):
                """P(w)[2 heads] = exp(S^T) * EM -- right after this phase's exp.

                Phases with h0 < PSPLIT go to VectorE, the rest to GpSimd.
                """
                p_t = st["p"]
                j = st["j"]
                es = st["es"]
                h0 = 2 * ph
                eng = nc.vector if h0 < PSPLIT else nc.gpsimd
                eng.tensor_tensor(
                    p_t[:, h0 : h0 + 2], es[:, h0 : h0 + 2], em_sb[:, j, h0 : h0 + 2], MUL
                )

            def pv_alloc(st):
                st["pvps"] = ps_pv.tile([96, 2, N], FP, tag="pv", name="pvps")
                st["dnps"] = ps_dn.tile([96, 2, N], FP, tag="dn", name="dnps")

            def pv_den_chunk(st, ph):
                """2 heads of PV + den matmuls for the previous window."""
                p_t = st["p"]
                vsb = st["vsb"]
                for hh in range(2):
                    h = 2 * ph + hh
                    rb = 32 * (h % 3)
                    fo = h // 3
                    for mt in range(2):
                        rhs = p_t[:, h, mt, :]
                        nc.tensor.matmul(
                            st["pvps"][rb : rb + 32, fo, :],
                            vsb[:, mt, D * h : D * (h + 1)],
                            rhs,
                            start=(mt == 0),
                            stop=(mt == 1),
                            tile_position=(0, rb),
                        )
                        nc.tensor.matmul(
                            st["dnps"][rb : rb + 32, fo, :],
                            ones16[:],
                            rhs,
                            start=(mt == 0),
                            stop=(mt == 1),
                            tile_position=(0, rb),
                        )

            def stage2b(w, st):
                """Normalize, project, and store window w (the previous one)."""
                pvps = st["pvps"]
                dnps = st["dnps"]
                ivd = wpool.tile([96, 2, N], FP, tag="ivd")
                nc.vector.reciprocal_approx_fast(ivd[:], dnps[:])
                aoT = wpool.tile([96, 2, N], F16, tag="aoT")
                nc.vector.tensor_tensor(aoT[:], pvps[:], ivd[:], MUL)
                yps = ps_m1.tile([128, 2, C], FP, tag="m1")
                for mt in range(2):
                    for j in range(2):
                        nc.tensor.matmul(
                            yps[:, mt, :],
                            aoT[:, j, 128 * mt : 128 * (mt + 1)],
                            wp_sb[:, j, :],
                            start=(j == 0),
                            stop=(j == 1),
                        )
                ysb = wpool.tile([128, 2, C], F16, tag="ysb")
                nc.vector.tensor_scalar_mul(ysb[:], yps[:], Y_DESCALE)
                nc.sync.dma_start(y_d[w], ysb[:])

            prev = None
            prev_w = None
            for it in range(WPC):
                cur = stage1a(it)
                p_alloc(cur)
                if prev is not None:
                    pv_alloc(prev)
                for ph in range(3):
                    if prev is not None:
                        pv_den_chunk(prev, ph)
                    scores_phase(cur, ph)
                    p_mult_chunk(cur, ph)
                if prev is not None:
                    stage2b(prev_w, prev)
                prev, prev_w = cur, it
            pv_alloc(prev)
            for ph in range(3):
                pv_den_chunk(prev, ph)
            stage2b(prev_w, prev)

    nc.compile()
    return nc


def _prep_inputs(inputs):
    x = np.asarray(inputs["x"], np.float32)
    mask = np.asarray(inputs["mask"], np.float32)
    w_qkv = np.asarray(inputs["w_qkv"], np.float32)
    b_qkv = np.asarray(inputs["b_qkv"], np.float32)
    w_proj = np.asarray(inputs["w_proj"], np.float32)
    b_proj = np.asarray(inputs["b_proj"], np.float32)
    H, W = int(inputs["H"]), int(inputs["W"])

    rpb = _pos_bias_host(
        H,
        W,
        *[
            np.asarray(inputs[k], np.float32)
            for k in (
                "pw0", "pb0", "g1", "be1", "w1", "b1",
                "g2", "be2", "w2", "b2", "g3", "be3", "w3", "b3",
            )
        ],
    )

    # EM[mb, h, m, n] = exp(mask[mb, n, m] + rpb[n, m, h]), device layout
    # [mb][p, h, mt, n] with m = 128*mt + p.
    bias = mask.transpose(0, 2, 1)[:, None] + rpb.transpose(2, 1, 0)[None]
    em = np.exp(bias).reshape(NMASK, HEADS, 2, 128, N).transpose(0, 3, 1, 2, 4)
    em = np.ascontiguousarray(em).astype(np.float16)  # [64, 128, 6, 2, 256]

    scale = float(D) ** -0.5
    wq = w_qkv[:, 0:C] * scale
    wk = w_qkv[:, C : 2 * C]
    wqk = np.zeros((C, 512), np.float32)
    wqk[:, 0:128] = wq[:, 0:128]
    wqk[:, 128:256] = wk[:, 0:128]
    wqk[:, 256:320] = wq[:, 128:C]
    wqk[:, 384:448] = wk[:, 128:C]
    wqk16 = np.zeros((2, 128, 512), np.float16)
    wqk16[0] = wqk[0:128]
    wqk16[1, 0:64] = wqk[128:C]
    wv16 = np.zeros((2, 128, C), np.float16)
    wv16[0] = w_qkv[0:128, 2 * C :]
    wv16[1, 0:64] = w_qkv[128:C, 2 * C :]
    wp16 = np.ascontiguousarray(
        w_proj.reshape(2, 96, C).transpose(1, 0, 2)
    ).astype(np.float16)

    # per-core x^T chunks [w, 2, 128, 256]
    xt16 = np.zeros((B, 2, 128, N), np.float16)
    xtr = x.transpose(0, 2, 1)  # [B, C, N]
    xt16[:, 0] = xtr[:, 0:128]
    xt16[:, 1, 0:64] = xtr[:, 128:C]

    in_maps = []
    for core in range(NCORES):
        bs = [_win_to_b(core, w) for w in range(WPC)]
        in_maps.append(
            {
                "xt": np.ascontiguousarray(xt16[bs]),
                "em": np.ascontiguousarray(em[MPC * core : MPC * (core + 1)]),
                "wqk": wqk16,
                "wv": wv16,
                "wp": wp16,
            }
        )
    # host-side constant correction for (generally zero) v/proj biases.
    ycorr = None
    if np.any(b_qkv != 0.0) or np.any(b_proj != 0.0):
        bv = b_qkv[2 * C :]
        ycorr = (bv @ w_proj + b_proj).astype(np.float32)
        if np.any(b_qkv[: 2 * C] != 0.0):
            raise NotImplementedError("nonzero q/k bias not supported by fast path")
    return in_maps, ycorr


def _assemble(results, ycorr):
    out = np.empty((B, N, C), np.float32)
    for core in range(NCORES):
        y = results[core]["y"]  # [WPC, 128, 2, C] fp16
        for w in range(WPC):
            b = _win_to_b(core, w)
            out[b] = y[w].transpose(1, 0, 2).reshape(N, C).astype(np.float32)
    if ycorr is not None:
        out += ycorr
    return out


def run(inputs, trace=False):
    from concourse.bass_utils import run_bass_kernel_spmd

    if "nc" not in _CACHE:
        _CACHE["nc"] = _build_nc()
    in_maps, ycorr = _prep_inputs(inputs)
    res = run_bass_kernel_spmd(
        _CACHE["nc"],
        in_maps,
        core_ids=list(range(NCORES)),
        trace=trace,
        trace_cores=[0] if trace else None,
    )
    return _assemble(res.results, ycorr), res


def get_nc():
    if "nc" not in _CACHE:
        _CACHE["nc"] = _build_nc()
    return _CACHE["nc"]


def kernel(**inputs):
    out, _ = run(inputs, trace=bool(int(os.environ.get("KERNEL_TRACE", "0"))))
    return out


# revision 17
# speedup vs baseline: 1.2217x; 1.0520x over previous
"""Trainium2 Bass kernel for nn_Attention_867583394433 (sparse window attention).

Strategy (8 NeuronCores, pure data parallel over windows B_=256 -> 32/core):
  - Host precomputes the tiny position-MLP -> relative-position-bias table and
    folds it with the additive mask into a multiplicative table
    EM[mask, head] = exp(rpb + mask) (fp16), laid out to match the on-chip
    transposed-score layout.  Windows are assigned so each core only touches
    8 distinct masks (mask index = b % 64) and EM stays SBUF-resident.
  - Device computes, per window, in transposed score layout S^T[m, n]
    (key index m on partitions, query index n on free dim):
      qk^T channel-layout matmuls -> scores (K=32, row-tiled)
      -> exp on ScalarE -> P = exp(S^T) * EM elementwise, split between
      VectorE and GpSimd (both only touch SBUF) to balance engines
      -> PV and denominator (ones-matmul, col-tiled into matching partition
         rows, packed [96, 2, 256]) on TensorE -> fast reciprocal + fused
         normalize on VectorE -> output projection -> fp16 DMA out.
  - The d^-0.5 attention scale is folded into wq on the host; the (zero)
    qkv/proj biases are folded into a host-side constant correction.
"""

import os

import numpy as np

HEADS = 6
D = 32
C = 192
N = 256
B = 256
NMASK = 64
POS_DIM = 12
EPS = 1e-5
NCORES = 8
WPC = B // NCORES  # 32 windows per core
MPC = NMASK // NCORES  # 8 masks per core
REP = B // NMASK  # 4 windows sharing one mask

S_AO = 16.0  # aoT = S_AO * attnout_true (via the ones value)
ONES_VAL = 1.0 / S_AO
Y_DESCALE = 1.0 / S_AO

# head split of the P = exp(S)*EM multiply: first PSPLIT heads on DVE,
# the rest on GpSimd.
PSPLIT = int(os.environ.get("PSPLIT", "2"))

_CACHE = {}


def _win_to_b(core, w):
    """Window order within a core: mask-major.  w = j*REP + k  ->  b."""
    j, k = divmod(w, REP)
    return NMASK * k + MPC * core + j


def _ln_np(x, g, b):
    m = x.mean(-1, keepdims=True)
    v = x.var(-1, keepdims=True)
    return (x - m) / np.sqrt(v + EPS) * g + b


def _pos_bias_host(H, W, pw0, pb0, g1, be1, w1, b1, g2, be2, w2, b2, g3, be3, w3, b3):
    """Replicates the reference position MLP + gather -> rpb [N, N, HEADS]."""
    H = int(H)
    W = int(W)
    ph = np.arange(1 - H, H)
    pw = np.arange(1 - W, W)
    biases = (
        np.stack(np.meshgrid(ph, pw, indexing="ij")).reshape(2, -1).T.astype(np.float32)
    )
    pos = biases @ pw0 + pb0
    pos = np.maximum(_ln_np(pos, g1, be1), 0.0) @ w1 + b1
    pos = np.maximum(_ln_np(pos, g2, be2), 0.0) @ w2 + b2
    pos = np.maximum(_ln_np(pos, g3, be3), 0.0) @ w3 + b3
    coords = np.stack(np.meshgrid(np.arange(H), np.arange(W), indexing="ij")).reshape(
        2, -1
    )
    rel = coords[:, :, None] - coords[:, None, :]
    rpi = (rel[0] + H - 1) * (2 * W - 1) + (rel[1] + W - 1)
    return pos[rpi]  # [N, N, HEADS] fp32


def _build_nc():
    import concourse.tile as tile
    from concourse import bacc, mybir

    FP = mybir.dt.float32
    F16 = mybir.dt.float16
    EXP = mybir.ActivationFunctionType.Exp
    COPY = mybir.ActivationFunctionType.Copy
    MUL = mybir.AluOpType.mult

    nc = bacc.Bacc("TRN2", target_bir_lowering=False, debug=False)
    # x^T feature chunks: [w, chunk(2), 128, 256]; chunk 1 holds features
    # 128:192 in rows 0:64 (rows 64:128 are zero padding).
    xt_d = nc.dram_tensor("xt", [WPC, 2, 128, N], F16, kind="ExternalInput")
    em_d = nc.dram_tensor("em", [MPC, 128, HEADS, 2, N], F16, kind="ExternalInput")
    wqk_d = nc.dram_tensor("wqk", [2, 128, 512], F16, kind="ExternalInput")
    wv_d = nc.dram_tensor("wv", [2, 128, C], F16, kind="ExternalInput")
    wp_d = nc.dram_tensor("wp", [96, 2, C], F16, kind="ExternalInput")
    y_d = nc.dram_tensor("y", [WPC, 128, 2, C], F16, kind="ExternalOutput")

    with tile.TileContext(nc) as tc:
        with (
            tc.tile_pool(name="const", bufs=1) as cpool,
            tc.tile_pool(name="win", bufs=int(os.environ.get("WBUFS", "2"))) as wpool,
            tc.tile_pool(name="big", bufs=int(os.environ.get("BBUFS", "3"))) as bpool,
            tc.tile_pool(name="pbig", bufs=int(os.environ.get("PBUFS", "2"))) as ppool,
            tc.tile_pool(name="ps_sc", bufs=2, space="PSUM") as ps_sc,
            tc.tile_pool(name="ps_m1", bufs=2, space="PSUM") as ps_m1,
            tc.tile_pool(name="ps_pv", bufs=1, space="PSUM") as ps_pv,
            tc.tile_pool(name="ps_dn", bufs=1, space="PSUM") as ps_dn,
        ):
            # ---- resident constants ----
            # (wv/wp/first-em DMAs are emitted from inside stage1a(0) so the
            # first window's x tile isn't queued behind them on the SP FIFO)
            em_sb = cpool.tile([128, MPC, HEADS, 2, N], F16)
            em_loaded = set()
            wqk_sb = cpool.tile([128, 2, 512], F16)
            nc.sync.dma_start(wqk_sb[:, 0], wqk_d[0])
            nc.sync.dma_start(wqk_sb[0:64, 1], wqk_d[1, 0:64])
            wv_sb = cpool.tile([128, 2, C], F16)
            wp_sb = cpool.tile([96, 2, C], F16)
            ones16 = cpool.tile([128, D], F16)
            nc.gpsimd.memset(ones16[:], ONES_VAL)

            # scores head -> (qk m-tile, partition row) maps
            q_loc = [(0, 32 * h) for h in range(4)] + [(2, 32 * (h - 4)) for h in (4, 5)]
            k_loc = [(1, 32 * h) for h in range(4)] + [(3, 32 * (h - 4)) for h in (4, 5)]

            def stage1a(w):
                """DMA x^T, qk matmuls + copy, v matmuls + copy, for window w."""
                j = w // REP
                xa = wpool.tile([128, 2, N], F16, tag="xa")
                nc.sync.dma_start(xa[:, 0, :], xt_d[w, 0])
                nc.sync.dma_start(xa[0:64, 1, :], xt_d[w, 1, 0:64])
                if w == 0:
                    nc.sync.dma_start(wv_sb[:, 0], wv_d[0])
                    nc.sync.dma_start(wv_sb[0:64, 1], wv_d[1, 0:64])
                for jl in {j, min((w + 2) // REP, MPC - 1)}:
                    if jl not in em_loaded:
                        em_loaded.add(jl)
                        nc.sync.dma_start(em_sb[:, jl], em_d[jl])
                if w == 0:
                    nc.sync.dma_start(wp_sb[:], wp_d[:])

                qkps = ps_sc.tile([128, 4, N], FP, tag="sc")
                for t in range(4):
                    nc.tensor.matmul(
                        qkps[:, t, :],
                        wqk_sb[:, 0, 128 * t : 128 * (t + 1)],
                        xa[:, 0, :],
                        start=True,
                        stop=False,
                    )
                    nc.tensor.matmul(
                        qkps[:, t, :],
                        wqk_sb[0:64, 1, 128 * t : 128 * (t + 1)],
                        xa[0:64, 1, :],
                        start=False,
                        stop=True,
                    )
                qkT = wpool.tile([128, 4, N], F16, tag="qkT")
                if int(os.environ.get("QKT_SPLIT", "1")):
                    nc.vector.tensor_copy(qkT[:, 0:2], qkps[:, 0:2])
                    nc.scalar.copy(qkT[:, 2:4], qkps[:, 2:4])
                else:
                    nc.vector.tensor_copy(qkT[:], qkps[:])

                vps = ps_m1.tile([128, 2, C], FP, tag="m1")
                for mt in range(2):
                    nc.tensor.matmul(
                        vps[:, mt, :],
                        xa[:, 0, 128 * mt : 128 * (mt + 1)],
                        wv_sb[:, 0, :],
                        start=True,
                        stop=False,
                    )
                    nc.tensor.matmul(
                        vps[:, mt, :],
                        xa[0:64, 1, 128 * mt : 128 * (mt + 1)],
                        wv_sb[0:64, 1, :],
                        start=False,
                        stop=True,
                    )
                vsb = wpool.tile([128, 2, C], F16, tag="vsb")
                nc.scalar.copy(vsb[:], vps[:])
                es = bpool.tile([128, HEADS, 2, N], F16, tag="es")
                return {"j": j, "qkT": qkT, "vsb": vsb, "es": es}

            def scores_phase(st, ph):
                """2 heads of S^T matmuls + one exp for this window."""
                qkT = st["qkT"]
                es = st["es"]
                scps = ps_sc.tile([128, 2, 2, N], FP, tag="sc")
                for hh in range(2):
                    h = 2 * ph + hh
                    qt, qr = q_loc[h]
                    kt, kr = k_loc[h]
                    for mt in range(2):
                        nc.tensor.matmul(
                            scps[:, hh, mt, :],
                            qkT[kr : kr + 32, kt, 128 * mt : 128 * (mt + 1)],
                            qkT[qr : qr + 32, qt, :],
                            start=True,
                            stop=True,
                            tile_position=(kr, 0),
                        )
                nc.scalar.activation(es[:, 2 * ph : 2 * ph + 2], scps[:], EXP)

            def p_alloc(st):
                st["p"] = ppool.tile([128, HEADS, 2, N], F16, tag="P", name="p_t")

            def p_mult_chunk(st, ph, force_dve=False):
                """P(w)[2 heads] = exp(S^T) * EM -- right after this phase's exp.

                Phases with h0 < PSPLIT go to VectorE, the rest to GpSimd.
                """
                p_t = st["p"]
                j = st["j"]
                es = st["es"]
                h0 = 2 * ph
                eng = nc.vector if (h0 < PSPLIT or force_dve) else nc.gpsimd
                eng.tensor_tensor(
                    p_t[:, h0 : h0 + 2], es[:, h0 : h0 + 2], em_sb[:, j, h0 : h0 + 2], MUL
                )

            def pv_alloc(st):
                st["pvps"] = ps_pv.tile([96, 2, N], FP, tag="pv", name="pvps")
                st["dnps"] = ps_dn.tile([96, 2, N], FP, tag="dn", name="dnps")

            def pv_den_chunk(st, ph):
                """2 heads of PV + den matmuls for the previous window."""
                p_t = st["p"]
                vsb = st["vsb"]
                for hh in range(2):
                    h = 2 * ph + hh
                    rb = 32 * (h % 3)
                    fo = h // 3
                    for mt in range(2):
                        rhs = p_t[:, h, mt, :]
                        nc.tensor.matmul(
                            st["pvps"][rb : rb + 32, fo, :],
                            vsb[:, mt, D * h : D * (h + 1)],
                            rhs,
                            start=(mt == 0),
                            stop=(mt == 1),
                            tile_position=(0, rb),
                        )
                        nc.tensor.matmul(
                            st["dnps"][rb : rb + 32, fo, :],
                            ones16[:],
                            rhs,
                            start=(mt == 0),
                            stop=(mt == 1),
                            tile_position=(0, rb),
                        )

            def stage2b(w, st):
                """Normalize, project, and store window w (the previous one)."""
                pvps = st["pvps"]
                dnps = st["dnps"]
                ivd = wpool.tile([96, 2, N], FP, tag="ivd")
                nc.vector.reciprocal_approx_fast(ivd[:], dnps[:])
                aoT = wpool.tile([96, 2, N], F16, tag="aoT")
                nc.vector.tensor_tensor(aoT[:], pvps[:], ivd[:], MUL)
                yps = ps_m1.tile([128, 2, C], FP, tag="m1")
                for mt in range(2):
                    for j in range(2):
                        nc.tensor.matmul(
                            yps[:, mt, :],
                            aoT[:, j, 128 * mt : 128 * (mt + 1)],
                            wp_sb[:, j, :],
                            start=(j == 0),
                            stop=(j == 1),
                        )
                ysb = wpool.tile([128, 2, C], F16, tag="ysb")
                nc.vector.tensor_scalar_mul(ysb[:], yps[:], Y_DESCALE)
                nc.sync.dma_start(y_d[w], ysb[:])

            prev = None
            prev_w = None
            for it in range(WPC):
                cur = stage1a(it)
                p_alloc(cur)
                if prev is not None:
                    pv_alloc(prev)
                for ph in range(3):
                    if prev is not None:
                        pv_den_chunk(prev, ph)
                    scores_phase(cur, ph)
                    p_mult_chunk(cur, ph, force_dve=(it == WPC - 1))
                if prev is not None:
                    stage2b(prev_w, prev)
                prev, prev_w = cur, it
            pv_alloc(prev)
            for ph in range(3):
                pv_den_chunk(prev, ph)
            stage2b(prev_w, prev)

    nc.compile()
    return nc


def _prep_inputs(inputs):
    x = np.asarray(inputs["x"], np.float32)
    mask = np.asarray(inputs["mask"], np.float32)
    w_qkv = np.asarray(inputs["w_qkv"], np.float32)
    b_qkv = np.asarray(inputs["b_qkv"], np.float32)
    w_proj = np.asarray(inputs["w_proj"], np.float32)
    b_proj = np.asarray(inputs["b_proj"], np.float32)
    H, W = int(inputs["H"]), int(inputs["W"])

    rpb = _pos_bias_host(
        H,
        W,
        *[
            np.asarray(inputs[k], np.float32)
            for k in (
                "pw0", "pb0", "g1", "be1", "w1", "b1",
                "g2", "be2", "w2", "b2", "g3", "be3", "w3", "b3",
            )
        ],
    )

    # EM[mb, h, m, n] = exp(mask[mb, n, m] + rpb[n, m, h]), device layout
    # [mb][p, h, mt, n] with m = 128*mt + p.
    bias = mask.transpose(0, 2, 1)[:, None] + rpb.transpose(2, 1, 0)[None]
    em = np.exp(bias).reshape(NMASK, HEADS, 2, 128, N).transpose(0, 3, 1, 2, 4)
    em = np.ascontiguousarray(em).astype(np.float16)  # [64, 128, 6, 2, 256]

    scale = float(D) ** -0.5
    wq = w_qkv[:, 0:C] * scale
    wk = w_qkv[:, C : 2 * C]
    wqk = np.zeros((C, 512), np.float32)
    wqk[:, 0:128] = wq[:, 0:128]
    wqk[:, 128:256] = wk[:, 0:128]
    wqk[:, 256:320] = wq[:, 128:C]
    wqk[:, 384:448] = wk[:, 128:C]
    wqk16 = np.zeros((2, 128, 512), np.float16)
    wqk16[0] = wqk[0:128]
    wqk16[1, 0:64] = wqk[128:C]
    wv16 = np.zeros((2, 128, C), np.float16)
    wv16[0] = w_qkv[0:128, 2 * C :]
    wv16[1, 0:64] = w_qkv[128:C, 2 * C :]
    wp16 = np.ascontiguousarray(
        w_proj.reshape(2, 96, C).transpose(1, 0, 2)
    ).astype(np.float16)

    # per-core x^T chunks [w, 2, 128, 256]
    xt16 = np.zeros((B, 2, 128, N), np.float16)
    xtr = x.transpose(0, 2, 1)  # [B, C, N]
    xt16[:, 0] = xtr[:, 0:128]
    xt16[:, 1, 0:64] = xtr[:, 128:C]

    in_maps = []
    for core in range(NCORES):
        bs = [_win_to_b(core, w) for w in range(WPC)]
        in_maps.append(
            {
                "xt": np.ascontiguousarray(xt16[bs]),
                "em": np.ascontiguousarray(em[MPC * core : MPC * (core + 1)]),
                "wqk": wqk16,
                "wv": wv16,
                "wp": wp16,
            }
        )
    # host-side constant correction for (generally zero) v/proj biases.
    ycorr = None
    if np.any(b_qkv != 0.0) or np.any(b_proj != 0.0):
        bv = b_qkv[2 * C :]
        ycorr = (bv @ w_proj + b_proj).astype(np.float32)
        if np.any(b_qkv[: 2 * C] != 0.0):
            raise NotImplementedError("nonzero q/k bias not supported by fast path")
    return in_maps, ycorr


def _assemble(results, ycorr):
    out = np.empty((B, N, C), np.float32)
    for core in range(NCORES):
        y = results[core]["y"]  # [WPC, 128, 2, C] fp16
        for w in range(WPC):
            b = _win_to_b(core, w)
            out[b] = y[w].transpose(1, 0, 2).reshape(N, C).astype(np.float32)
    if ycorr is not None:
        out += ycorr
    return out


def run(inputs, trace=False):
    from concourse.bass_utils import run_bass_kernel_spmd

    if "nc" not in _CACHE:
        _CACHE["nc"] = _build_nc()
    in_maps, ycorr = _prep_inputs(inputs)
    res = run_bass_kernel_spmd(
        _CACHE["nc"],
        in_maps,
        core_ids=list(range(NCORES)),
        trace=trace,
        trace_cores=[0] if trace else None,
    )
    return _assemble(res.results, ycorr), res


def get_nc():
    if "nc" not in _CACHE:
        _CACHE["nc"] = _build_nc()
    return _CACHE["nc"]


def kernel(**inputs):
    out, _ = run(inputs, trace=bool(int(os.environ.get("KERNEL_TRACE", "0"))))
    return out


# revision 25
# speedup vs baseline: 1.5644x; 1.2805x over previous
"""Trainium2 Bass kernel for nn_Attention_867583394433 (sparse window attention).

Strategy (8 NeuronCores, pure data parallel over windows B_=256 -> 32/core):
  - Host precomputes the tiny position-MLP -> relative-position-bias table and
    folds it with the additive mask into a multiplicative table
    EM[mask, head] = exp(rpb + mask) (fp16), laid out to match the on-chip
    transposed-score layout.  Windows are assigned so each core only touches
    8 distinct masks (mask index = b % 64) and EM stays SBUF-resident.
  - Device computes, per window, in transposed score layout S^T[m, n]
    (key index m on partitions, query index n on free dim):
      qk^T channel-layout matmuls -> scores (K=32, row-tiled)
      -> exp on ScalarE -> P = exp(S^T) * EM elementwise, split between
      VectorE and GpSimd (both only touch SBUF) to balance engines
      -> PV and denominator (ones-matmul, col-tiled into matching partition
         rows, packed [96, 2, 256]) on TensorE -> fast reciprocal + fused
         normalize on VectorE -> output projection -> fp16 DMA out.
  - The d^-0.5 attention scale is folded into wq on the host; the (zero)
    qkv/proj biases are folded into a host-side constant correction.
"""

import os

import numpy as np

HEADS = 6
D = 32
C = 192
N = 256
B = 256
NMASK = 64
POS_DIM = 12
EPS = 1e-5
NCORES = 8
WPC = B // NCORES  # 32 windows per core
MPC = NMASK // NCORES  # 8 masks per core
REP = B // NMASK  # 4 windows sharing one mask

S_AO = 16.0  # aoT = S_AO * attnout_true (via the ones value)
ONES_VAL = 1.0 / S_AO
Y_DESCALE = 1.0 / S_AO

# head split of the P = exp(S)*EM multiply: first PSPLIT heads on DVE,
# the rest on GpSimd.
PSPLIT = int(os.environ.get("PSPLIT", "3"))

_CACHE = {}


def _win_to_b(core, w):
    """Window order within a core: mask-major.  w = j*REP + k  ->  b."""
    j, k = divmod(w, REP)
    return NMASK * k + MPC * core + j


def _ln_np(x, g, b):
    m = x.mean(-1, keepdims=True)
    v = x.var(-1, keepdims=True)
    return (x - m) / np.sqrt(v + EPS) * g + b


def _pos_bias_host(H, W, pw0, pb0, g1, be1, w1, b1, g2, be2, w2, b2, g3, be3, w3, b3):
    """Replicates the reference position MLP + gather -> rpb [N, N, HEADS]."""
    H = int(H)
    W = int(W)
    ph = np.arange(1 - H, H)
    pw = np.arange(1 - W, W)
    biases = (
        np.stack(np.meshgrid(ph, pw, indexing="ij")).reshape(2, -1).T.astype(np.float32)
    )
    pos = biases @ pw0 + pb0
    pos = np.maximum(_ln_np(pos, g1, be1), 0.0) @ w1 + b1
    pos = np.maximum(_ln_np(pos, g2, be2), 0.0) @ w2 + b2
    pos = np.maximum(_ln_np(pos, g3, be3), 0.0) @ w3 + b3
    coords = np.stack(np.meshgrid(np.arange(H), np.arange(W), indexing="ij")).reshape(
        2, -1
    )
    rel = coords[:, :, None] - coords[:, None, :]
    rpi = (rel[0] + H - 1) * (2 * W - 1) + (rel[1] + W - 1)
    return pos[rpi]  # [N, N, HEADS] fp32


def _build_nc():
    import concourse.tile as tile
    from concourse import bacc, mybir

    FP = mybir.dt.float32
    F16 = mybir.dt.float16
    EXP = mybir.ActivationFunctionType.Exp
    COPY = mybir.ActivationFunctionType.Copy
    MUL = mybir.AluOpType.mult

    nc = bacc.Bacc("TRN2", target_bir_lowering=False, debug=False)
    # x^T feature chunks: [w, chunk(2), 128, 256]; chunk 1 holds features
    # 128:192 in rows 0:64 (rows 64:128 are zero padding).
    xt_d = nc.dram_tensor("xt", [WPC, 2, 128, N], F16, kind="ExternalInput")
    em_d = nc.dram_tensor("em", [MPC, 128, HEADS, 2, N], F16, kind="ExternalInput")
    wqk_d = nc.dram_tensor("wqk", [2, 128, 512], F16, kind="ExternalInput")
    wv_d = nc.dram_tensor("wv", [2, 128, C], F16, kind="ExternalInput")
    wp_d = nc.dram_tensor("wp", [2, 128, C], F16, kind="ExternalInput")
    id_d = nc.dram_tensor("ident", [128, 128], F16, kind="ExternalInput")
    y_d = nc.dram_tensor("y", [WPC, 128, 2, C], F16, kind="ExternalOutput")

    with tile.TileContext(nc) as tc:
        with (
            tc.tile_pool(name="const", bufs=1) as cpool,
            tc.tile_pool(name="win", bufs=int(os.environ.get("WBUFS", "2"))) as wpool,
            tc.tile_pool(name="big", bufs=int(os.environ.get("BBUFS", "3"))) as bpool,
            tc.tile_pool(name="pbig", bufs=int(os.environ.get("PBUFS", "2"))) as ppool,
            tc.tile_pool(name="ps_qk", bufs=int(os.environ.get("QKBUFS", "1")), space="PSUM") as ps_qk,
            tc.tile_pool(name="ps_sc", bufs=2, space="PSUM") as ps_sc,
            tc.tile_pool(name="ps_m1", bufs=2, space="PSUM") as ps_m1,
            tc.tile_pool(name="ps_yo", bufs=int(os.environ.get("YOBUFS", "1")), space="PSUM") as ps_yo,
        ):
            # ---- resident constants ----
            # (wv/wp/first-em DMAs are emitted from inside stage1a(0) so the
            # first window's x tile isn't queued behind them on the SP FIFO)
            em_sb = cpool.tile([128, MPC, HEADS, 2, N], F16)
            em_loaded = set()
            wqk_sb = cpool.tile([128, 2, 512], F16)
            nc.sync.dma_start(wqk_sb[:, 0], wqk_d[0])
            nc.sync.dma_start(wqk_sb[0:64, 1], wqk_d[1, 0:64])
            wv_sb = cpool.tile([128, 2, C], F16)
            wp_sb = cpool.tile([128, 2, C], F16)
            id_sb = cpool.tile([128, 128], F16)

            # scores head -> (qk m-tile, partition row) maps
            q_loc = [(0, 32 * h) for h in range(4)] + [(2, 32 * (h - 4)) for h in (4, 5)]
            k_loc = [(1, 32 * h) for h in range(4)] + [(3, 32 * (h - 4)) for h in (4, 5)]

            def stage1a(w):
                """DMA x^T, qk matmuls + copy, v matmuls + copy, for window w."""
                j = w // REP
                xa = wpool.tile([128, 2, N], F16, tag="xa")
                nc.sync.dma_start(xa[:, 0, :], xt_d[w, 0])
                nc.sync.dma_start(xa[0:64, 1, :], xt_d[w, 1, 0:64])
                if w == 0:
                    nc.sync.dma_start(wv_sb[:, 0], wv_d[0])
                    nc.sync.dma_start(wv_sb[0:64, 1], wv_d[1, 0:64])
                for jl in {j, min((w + 2) // REP, MPC - 1)}:
                    if jl not in em_loaded:
                        em_loaded.add(jl)
                        nc.sync.dma_start(em_sb[:, jl], em_d[jl])
                if w == 0:
                    nc.sync.dma_start(wp_sb[:, 0], wp_d[0])
                    nc.sync.dma_start(wp_sb[0:64, 1], wp_d[1, 0:64])
                    nc.sync.dma_start(id_sb[:], id_d[:])

                qkT = wpool.tile([128, 4, N], F16, tag="qkT")
                qhalf = []
                for half in range(2):
                    qkps = ps_qk.tile(
                        [128, 2, N], FP, tag="qk", name="qkps"
                    )
                    qhalf.append(qkps)
                    for tt in range(2):
                        t = 2 * half + tt
                        nc.tensor.matmul(
                            qkps[:, tt, :],
                            wqk_sb[:, 0, 128 * t : 128 * (t + 1)],
                            xa[:, 0, :],
                            start=True,
                            stop=False,
                        )
                        nc.tensor.matmul(
                            qkps[:, tt, :],
                            wqk_sb[0:64, 1, 128 * t : 128 * (t + 1)],
                            xa[0:64, 1, :],
                            start=False,
                            stop=True,
                        )
                if int(os.environ.get("QKT_SPLIT", "0")):
                    nc.vector.tensor_copy(qkT[:, 0:2], qhalf[0][:])
                    nc.scalar.copy(qkT[:, 2:4], qhalf[1][:])
                else:
                    nc.vector.tensor_copy(qkT[:, 0:2], qhalf[0][:])
                    nc.vector.tensor_copy(qkT[:, 2:4], qhalf[1][:])

                vps = ps_m1.tile([128, 2, C], FP, tag="m1")
                for mt in range(2):
                    nc.tensor.matmul(
                        vps[:, mt, :],
                        xa[:, 0, 128 * mt : 128 * (mt + 1)],
                        wv_sb[:, 0, :],
                        start=True,
                        stop=False,
                    )
                    nc.tensor.matmul(
                        vps[:, mt, :],
                        xa[0:64, 1, 128 * mt : 128 * (mt + 1)],
                        wv_sb[0:64, 1, :],
                        start=False,
                        stop=True,
                    )
                es = bpool.tile([128, HEADS, 2, N], F16, tag="es")
                return {"j": j, "qkT": qkT, "vps": vps, "es": es}

            def vaug_copy(st):
                vaug = wpool.tile([128, 2, HEADS, D + 1], F16, tag="vaug")
                nc.vector.tensor_copy(
                    vaug[:, :, :, 0:D],
                    st["vps"][:].rearrange("p mt (h d) -> p mt h d", h=HEADS),
                )
                nc.gpsimd.memset(vaug[:, :, :, D : D + 1], ONES_VAL)
                st["vaug"] = vaug

            def scores_phase(st, ph):
                """2 heads of S^T matmuls + one exp for this window."""
                qkT = st["qkT"]
                es = st["es"]
                scps = ps_sc.tile([128, 2, 2, N], FP, tag="sc", name="scps")
                for hh in range(2):
                    h = 2 * ph + hh
                    qt, qr = q_loc[h]
                    kt, kr = k_loc[h]
                    for mt in range(2):
                        nc.tensor.matmul(
                            scps[:, hh, mt, :],
                            qkT[kr : kr + 32, kt, 128 * mt : 128 * (mt + 1)],
                            qkT[qr : qr + 32, qt, :],
                            start=True,
                            stop=True,
                            tile_position=(kr, 0),
                        )
                nc.scalar.activation(es[:, 2 * ph : 2 * ph + 2], scps[:], EXP)

            def p_alloc(st):
                st["p"] = ppool.tile([128, HEADS, 2, N], F16, tag="P", name="p_t")

            def p_mult_chunk(st, h, force_dve=False):
                """P(w)[head h] = exp(S^T) * EM -- right after this head's exp.

                Heads below PSPLIT go to VectorE, the rest to GpSimd.
                """
                p_t = st["p"]
                j = st["j"]
                es = st["es"]
                eng = nc.vector if (h < PSPLIT or force_dve) else nc.gpsimd
                eng.tensor_tensor(
                    p_t[:, h], es[:, h], em_sb[:, j, h], MUL
                )

            def pv_alloc(st):
                st["yo"] = ps_yo.tile([128, 2, HEADS, D + 1], FP, tag="yo", name="yo")

            def pv_den_chunk(st, h):
                """One head of transposed PV+den matmuls: P-block stationary,
                [v | ones] moving -> out [128 queries, 33] (ap_size 33)."""
                p_t = st["p"]
                vaug = st["vaug"]
                for ts in range(2):
                    for mt in range(2):
                        nc.tensor.matmul(
                            st["yo"][:, ts, h, :],
                            p_t[:, h, mt, 128 * ts : 128 * (ts + 1)],
                            vaug[:, mt, h, :],
                            start=(mt == 0),
                            stop=(mt == 1),
                        )

            def stage2b_a(st):
                """Normalize (broadcast divide) on VectorE -- emitted early."""
                yo = st["yo"]
                ivd = wpool.tile([128, 2, HEADS, 1], FP, tag="ivd")
                nc.vector.reciprocal_approx_fast(
                    ivd[:].rearrange("p a b one -> p (a b) (one)"),
                    yo[:, :, :, D : D + 1].rearrange("p a b one -> p (a b) (one)"),
                )
                aoU = wpool.tile([128, 2, HEADS, D], F16, tag="aoU")
                nc.vector.tensor_tensor(
                    aoU[:], yo[:, :, :, 0:D], ivd[:].broadcast_to([128, 2, HEADS, D]), MUL
                )
                st["aoU"] = aoU

            def stage2b_b(st):
                """Transpose + project."""
                aoU = st["aoU"]
                aot_ps = ps_m1.tile([128, 2, 2, 128], F16, tag="m1", name="aot_ps")
                for ts in range(2):
                    nc.tensor.transpose(
                        aot_ps[:, ts, 0, :],
                        aoU[:, ts, 0:4, :].rearrange("p h d -> p (h d)"),
                        id_sb[:],
                    )
                    nc.tensor.transpose(
                        aot_ps[0:64, ts, 1, :],
                        aoU[:, ts, 4:6, :].rearrange("p h d -> p (h d)"),
                        id_sb[:],
                    )
                aot = wpool.tile([128, 2, 2, 128], F16, tag="aots")
                nc.vector.tensor_copy(aot[:], aot_ps[:])
                yps = ps_m1.tile([128, 2, C], FP, tag="m1")
                for ts in range(2):
                    nc.tensor.matmul(
                        yps[:, ts, :],
                        aot[:, ts, 0, :],
                        wp_sb[:, 0, :],
                        start=True,
                        stop=False,
                    )
                    nc.tensor.matmul(
                        yps[:, ts, :],
                        aot[0:64, ts, 1, :],
                        wp_sb[0:64, 1, :],
                        start=False,
                        stop=True,
                    )
                st["yps"] = yps

            def stage2b_c(w, st):
                """Descale + store."""
                ysb = wpool.tile([128, 2, C], F16, tag="ysb")
                nc.scalar.activation(ysb[:], st["yps"], COPY, scale=Y_DESCALE)
                nc.sync.dma_start(y_d[w], ysb[:])

            window = []  # (w, state) pending retirement, newest last
            for it in range(WPC):
                cur = stage1a(it)
                p_alloc(cur)
                old = window.pop(0) if len(window) >= 2 else None
                if old is not None:
                    stage2b_a(old[1])
                if window:
                    pv_alloc(window[-1][1])
                for ph in range(3):
                    if window:
                        pv_den_chunk(window[-1][1], 2 * ph)
                        pv_den_chunk(window[-1][1], 2 * ph + 1)
                    scores_phase(cur, ph)
                    p_mult_chunk(cur, 2 * ph, force_dve=(it >= WPC - 2))
                    p_mult_chunk(cur, 2 * ph + 1, force_dve=(it >= WPC - 2))
                    if ph == 1:
                        if old is not None:
                            stage2b_b(old[1])
                        vaug_copy(cur)
                if old is not None:
                    stage2b_c(*old)
                window.append((it, cur))
            last_w, last = window[-1]
            pv_alloc(last)
            for h in range(HEADS):
                pv_den_chunk(last, h)
            for w, st in window:
                stage2b_a(st)
                stage2b_b(st)
                stage2b_c(w, st)

    nc.compile()
    return nc


def _prep_inputs(inputs):
    x = np.asarray(inputs["x"], np.float32)
    mask = np.asarray(inputs["mask"], np.float32)
    w_qkv = np.asarray(inputs["w_qkv"], np.float32)
    b_qkv = np.asarray(inputs["b_qkv"], np.float32)
    w_proj = np.asarray(inputs["w_proj"], np.float32)
    b_proj = np.asarray(inputs["b_proj"], np.float32)
    H, W = int(inputs["H"]), int(inputs["W"])

    rpb = _pos_bias_host(
        H,
        W,
        *[
            np.asarray(inputs[k], np.float32)
            for k in (
                "pw0", "pb0", "g1", "be1", "w1", "b1",
                "g2", "be2", "w2", "b2", "g3", "be3", "w3", "b3",
            )
        ],
    )

    # EM[mb, h, m, n] = exp(mask[mb, n, m] + rpb[n, m, h]), device layout
    # [mb][p, h, mt, n] with m = 128*mt + p.
    bias = mask.transpose(0, 2, 1)[:, None] + rpb.transpose(2, 1, 0)[None]
    em = np.exp(bias).reshape(NMASK, HEADS, 2, 128, N).transpose(0, 3, 1, 2, 4)
    em = np.ascontiguousarray(em).astype(np.float16)  # [64, 128, 6, 2, 256]

    scale = float(D) ** -0.5
    wq = w_qkv[:, 0:C] * scale
    wk = w_qkv[:, C : 2 * C]
    wqk = np.zeros((C, 512), np.float32)
    wqk[:, 0:128] = wq[:, 0:128]
    wqk[:, 128:256] = wk[:, 0:128]
    wqk[:, 256:320] = wq[:, 128:C]
    wqk[:, 384:448] = wk[:, 128:C]
    wqk16 = np.zeros((2, 128, 512), np.float16)
    wqk16[0] = wqk[0:128]
    wqk16[1, 0:64] = wqk[128:C]
    wv16 = np.zeros((2, 128, C), np.float16)
    wv16[0] = w_qkv[0:128, 2 * C :]
    wv16[1, 0:64] = w_qkv[128:C, 2 * C :]
    wp16 = np.zeros((2, 128, C), np.float16)
    wp16[0] = w_proj[0:128]
    wp16[1, 0:64] = w_proj[128:C]
    ident = np.eye(128, dtype=np.float16)

    # per-core x^T chunks [w, 2, 128, 256]
    xt16 = np.zeros((B, 2, 128, N), np.float16)
    xtr = x.transpose(0, 2, 1)  # [B, C, N]
    xt16[:, 0] = xtr[:, 0:128]
    xt16[:, 1, 0:64] = xtr[:, 128:C]

    in_maps = []
    for core in range(NCORES):
        bs = [_win_to_b(core, w) for w in range(WPC)]
        in_maps.append(
            {
                "xt": np.ascontiguousarray(xt16[bs]),
                "em": np.ascontiguousarray(em[MPC * core : MPC * (core + 1)]),
                "wqk": wqk16,
                "wv": wv16,
                "wp": wp16,
                "ident": ident,
            }
        )
    # host-side constant correction for (generally zero) v/proj biases.
    ycorr = None
    if np.any(b_qkv != 0.0) or np.any(b_proj != 0.0):
        bv = b_qkv[2 * C :]
        ycorr = (bv @ w_proj + b_proj).astype(np.float32)
        if np.any(b_qkv[: 2 * C] != 0.0):
            raise NotImplementedError("nonzero q/k bias not supported by fast path")
    return in_maps, ycorr


def _assemble(results, ycorr):
    out = np.empty((B, N, C), np.float32)
    for core in range(NCORES):
        y = results[core]["y"]  # [WPC, 128, 2, C] fp16
        for w in range(WPC):
            b = _win_to_b(core, w)
            out[b] = y[w].transpose(1, 0, 2).reshape(N, C).astype(np.float32)
    if ycorr is not None:
        out += ycorr
    return out


def run(inputs, trace=False):
    from concourse.bass_utils import run_bass_kernel_spmd

    if "nc" not in _CACHE:
        _CACHE["nc"] = _build_nc()
    in_maps, ycorr = _prep_inputs(inputs)
    res = run_bass_kernel_spmd(
        _CACHE["nc"],
        in_maps,
        core_ids=list(range(NCORES)),
        trace=trace,
        trace_cores=[0] if trace else None,
    )
    return _assemble(res.results, ycorr), res


def get_nc():
    if "nc" not in _CACHE:
        _CACHE["nc"] = _build_nc()
    return _CACHE["nc"]


def kernel(**inputs):
    out, _ = run(inputs, trace=bool(int(os.environ.get("KERNEL_TRACE", "0"))))
    return out


# revision 27
# speedup vs baseline: 1.5645x; 1.0001x over previous
"""Trainium2 Bass kernel for nn_Attention_867583394433 (sparse window attention).

Strategy (8 NeuronCores, pure data parallel over windows B_=256 -> 32/core):
  - Host precomputes the tiny position-MLP -> relative-position-bias table and
    folds it with the additive mask into a multiplicative table
    EM[mask, head] = exp(rpb + mask) (fp16), laid out to match the on-chip
    transposed-score layout.  Windows are assigned so each core only touches
    8 distinct masks (mask index = b % 64) and EM stays SBUF-resident.
  - Device computes, per window, in transposed score layout S^T[m, n]
    (key index m on partitions, query index n on free dim):
      qk^T channel-layout matmuls -> scores (K=32, row-tiled)
      -> exp on ScalarE -> P = exp(S^T) * EM elementwise, split between
      VectorE and GpSimd (both only touch SBUF) to balance engines
      -> PV and denominator (ones-matmul, col-tiled into matching partition
         rows, packed [96, 2, 256]) on TensorE -> fast reciprocal + fused
         normalize on VectorE -> output projection -> fp16 DMA out.
  - The d^-0.5 attention scale is folded into wq on the host; the (zero)
    qkv/proj biases are folded into a host-side constant correction.
"""

import os

import numpy as np

HEADS = 6
D = 32
C = 192
N = 256
B = 256
NMASK = 64
POS_DIM = 12
EPS = 1e-5
NCORES = 8
WPC = B // NCORES  # 32 windows per core
MPC = NMASK // NCORES  # 8 masks per core
REP = B // NMASK  # 4 windows sharing one mask

S_AO = 16.0  # aoT = S_AO * attnout_true (via the ones value)
ONES_VAL = 1.0 / S_AO
Y_DESCALE = 1.0 / S_AO

# head split of the P = exp(S)*EM multiply: first PSPLIT heads on DVE,
# the rest on GpSimd.
PSPLIT = int(os.environ.get("PSPLIT", "3"))

_CACHE = {}


def _win_to_b(core, w):
    """Window order within a core: mask-major.  w = j*REP + k  ->  b."""
    j, k = divmod(w, REP)
    return NMASK * k + MPC * core + j


def _ln_np(x, g, b):
    m = x.mean(-1, keepdims=True)
    v = x.var(-1, keepdims=True)
    return (x - m) / np.sqrt(v + EPS) * g + b


def _pos_bias_host(H, W, pw0, pb0, g1, be1, w1, b1, g2, be2, w2, b2, g3, be3, w3, b3):
    """Replicates the reference position MLP + gather -> rpb [N, N, HEADS]."""
    H = int(H)
    W = int(W)
    ph = np.arange(1 - H, H)
    pw = np.arange(1 - W, W)
    biases = (
        np.stack(np.meshgrid(ph, pw, indexing="ij")).reshape(2, -1).T.astype(np.float32)
    )
    pos = biases @ pw0 + pb0
    pos = np.maximum(_ln_np(pos, g1, be1), 0.0) @ w1 + b1
    pos = np.maximum(_ln_np(pos, g2, be2), 0.0) @ w2 + b2
    pos = np.maximum(_ln_np(pos, g3, be3), 0.0) @ w3 + b3
    coords = np.stack(np.meshgrid(np.arange(H), np.arange(W), indexing="ij")).reshape(
        2, -1
    )
    rel = coords[:, :, None] - coords[:, None, :]
    rpi = (rel[0] + H - 1) * (2 * W - 1) + (rel[1] + W - 1)
    return pos[rpi]  # [N, N, HEADS] fp32


def _build_nc():
    import concourse.tile as tile
    from concourse import bacc, mybir

    FP = mybir.dt.float32
    F16 = mybir.dt.float16
    EXP = mybir.ActivationFunctionType.Exp
    COPY = mybir.ActivationFunctionType.Copy
    MUL = mybir.AluOpType.mult

    nc = bacc.Bacc("TRN2", target_bir_lowering=False, debug=False)
    # x^T feature chunks: [w, chunk(2), 128, 256]; chunk 1 holds features
    # 128:192 in rows 0:64 (rows 64:128 are zero padding).
    xt_d = nc.dram_tensor("xt", [WPC, 2, 128, N], F16, kind="ExternalInput")
    em_d = nc.dram_tensor("em", [MPC, 128, HEADS, 2, N], F16, kind="ExternalInput")
    wqk_d = nc.dram_tensor("wqk", [2, 128, 512], F16, kind="ExternalInput")
    wv_d = nc.dram_tensor("wv", [2, 128, C], F16, kind="ExternalInput")
    wp_d = nc.dram_tensor("wp", [2, 128, C], F16, kind="ExternalInput")
    id_d = nc.dram_tensor("ident", [128, 128], F16, kind="ExternalInput")
    y_d = nc.dram_tensor("y", [WPC, 128, 2, C], F16, kind="ExternalOutput")

    with tile.TileContext(nc) as tc:
        with (
            tc.tile_pool(name="const", bufs=1) as cpool,
            tc.tile_pool(name="win", bufs=int(os.environ.get("WBUFS", "2"))) as wpool,
            tc.tile_pool(name="big", bufs=int(os.environ.get("BBUFS", "3"))) as bpool,
            tc.tile_pool(name="pbig", bufs=int(os.environ.get("PBUFS", "2"))) as ppool,
            tc.tile_pool(name="ps_qk", bufs=int(os.environ.get("QKBUFS", "1")), space="PSUM") as ps_qk,
            tc.tile_pool(name="ps_sc", bufs=2, space="PSUM") as ps_sc,
            tc.tile_pool(name="ps_m1", bufs=2, space="PSUM") as ps_m1,
            tc.tile_pool(name="ps_yo", bufs=int(os.environ.get("YOBUFS", "1")), space="PSUM") as ps_yo,
        ):
            # ---- resident constants ----
            # (wv/wp/first-em DMAs are emitted from inside stage1a(0) so the
            # first window's x tile isn't queued behind them on the SP FIFO)
            em_sb = cpool.tile([128, MPC, HEADS, 2, N], F16)
            em_loaded = set()
            wqk_sb = cpool.tile([128, 2, 512], F16)
            nc.sync.dma_start(wqk_sb[:, 0], wqk_d[0])
            nc.sync.dma_start(wqk_sb[0:64, 1], wqk_d[1, 0:64])
            wv_sb = cpool.tile([128, 2, C], F16)
            wp_sb = cpool.tile([128, 2, C], F16)
            id_sb = cpool.tile([128, 128], F16)
            vaug_ring = cpool.tile([128, 2, 2, HEADS, D + 1], F16)
            nc.gpsimd.memset(vaug_ring[:, :, :, :, D : D + 1], ONES_VAL)

            # scores head -> (qk m-tile, partition row) maps
            q_loc = [(0, 32 * h) for h in range(4)] + [(2, 32 * (h - 4)) for h in (4, 5)]
            k_loc = [(1, 32 * h) for h in range(4)] + [(3, 32 * (h - 4)) for h in (4, 5)]

            def stage1a(w):
                """DMA x^T, qk matmuls + copy, v matmuls + copy, for window w."""
                j = w // REP
                xa = wpool.tile([128, 2, N], F16, tag="xa")
                nc.sync.dma_start(xa[:, 0, :], xt_d[w, 0])
                nc.sync.dma_start(xa[0:64, 1, :], xt_d[w, 1, 0:64])
                if w == 0:
                    nc.sync.dma_start(wv_sb[:, 0], wv_d[0])
                    nc.sync.dma_start(wv_sb[0:64, 1], wv_d[1, 0:64])
                for jl in {j, min((w + 2) // REP, MPC - 1)}:
                    if jl not in em_loaded:
                        em_loaded.add(jl)
                        nc.sync.dma_start(em_sb[:, jl], em_d[jl])
                if w == 0:
                    nc.sync.dma_start(wp_sb[:, 0], wp_d[0])
                    nc.sync.dma_start(wp_sb[0:64, 1], wp_d[1, 0:64])
                    nc.sync.dma_start(id_sb[:], id_d[:])

                qkT = wpool.tile([128, 4, N], F16, tag="qkT")
                qhalf = []
                for half in range(2):
                    qkps = ps_qk.tile(
                        [128, 2, N], FP, tag="qk", name="qkps"
                    )
                    qhalf.append(qkps)
                    for tt in range(2):
                        t = 2 * half + tt
                        nc.tensor.matmul(
                            qkps[:, tt, :],
                            wqk_sb[:, 0, 128 * t : 128 * (t + 1)],
                            xa[:, 0, :],
                            start=True,
                            stop=False,
                        )
                        nc.tensor.matmul(
                            qkps[:, tt, :],
                            wqk_sb[0:64, 1, 128 * t : 128 * (t + 1)],
                            xa[0:64, 1, :],
                            start=False,
                            stop=True,
                        )
                if int(os.environ.get("QKT_SPLIT", "0")):
                    nc.vector.tensor_copy(qkT[:, 0:2], qhalf[0][:])
                    nc.scalar.copy(qkT[:, 2:4], qhalf[1][:])
                else:
                    nc.vector.tensor_copy(qkT[:, 0:2], qhalf[0][:])
                    nc.vector.tensor_copy(qkT[:, 2:4], qhalf[1][:])

                vps = ps_m1.tile([128, 2, C], FP, tag="m1")
                for mt in range(2):
                    nc.tensor.matmul(
                        vps[:, mt, :],
                        xa[:, 0, 128 * mt : 128 * (mt + 1)],
                        wv_sb[:, 0, :],
                        start=True,
                        stop=False,
                    )
                    nc.tensor.matmul(
                        vps[:, mt, :],
                        xa[0:64, 1, 128 * mt : 128 * (mt + 1)],
                        wv_sb[0:64, 1, :],
                        start=False,
                        stop=True,
                    )
                es = bpool.tile([128, HEADS, 2, N], F16, tag="es")
                return {"j": j, "qkT": qkT, "vps": vps, "es": es}

            def vaug_copy(st, w):
                vaug = vaug_ring[:, w % 2]
                nc.vector.tensor_copy(
                    vaug[:, :, :, 0:D],
                    st["vps"][:].rearrange("p mt (h d) -> p mt h d", h=HEADS),
                )
                st["vaug"] = vaug

            def scores_phase(st, ph):
                """2 heads of S^T matmuls + one exp for this window."""
                qkT = st["qkT"]
                es = st["es"]
                scps = ps_sc.tile([128, 2, 2, N], FP, tag="sc", name="scps")
                for hh in range(2):
                    h = 2 * ph + hh
                    qt, qr = q_loc[h]
                    kt, kr = k_loc[h]
                    for mt in range(2):
                        nc.tensor.matmul(
                            scps[:, hh, mt, :],
                            qkT[kr : kr + 32, kt, 128 * mt : 128 * (mt + 1)],
                            qkT[qr : qr + 32, qt, :],
                            start=True,
                            stop=True,
                            tile_position=(kr, 0),
                        )
                nc.scalar.activation(es[:, 2 * ph : 2 * ph + 2], scps[:], EXP)

            def p_alloc(st):
                st["p"] = ppool.tile([128, HEADS, 2, N], F16, tag="P", name="p_t")

            def p_mult_group(st, h0, h1, eng):
                """P(w)[h0:h1] = exp(S^T) * EM as one elementwise op."""
                p_t = st["p"]
                j = st["j"]
                es = st["es"]
                eng.tensor_tensor(
                    p_t[:, h0:h1], es[:, h0:h1], em_sb[:, j, h0:h1], MUL
                )

            def pv_alloc(st):
                st["yo"] = ps_yo.tile([128, 2, HEADS, D + 1], FP, tag="yo", name="yo")

            def pv_den_chunk(st, h):
                """One head of transposed PV+den matmuls: P-block stationary,
                [v | ones] moving -> out [128 queries, 33] (ap_size 33)."""
                p_t = st["p"]
                vaug = st["vaug"]
                for ts in range(2):
                    for mt in range(2):
                        nc.tensor.matmul(
                            st["yo"][:, ts, h, :],
                            p_t[:, h, mt, 128 * ts : 128 * (ts + 1)],
                            vaug[:, mt, h, :],
                            start=(mt == 0),
                            stop=(mt == 1),
                        )

            def stage2b_a(st):
                """Normalize (broadcast divide) on VectorE -- emitted early."""
                yo = st["yo"]
                ivd = wpool.tile([128, 2, HEADS, 1], FP, tag="ivd")
                nc.vector.reciprocal_approx_fast(
                    ivd[:].rearrange("p a b one -> p (a b) (one)"),
                    yo[:, :, :, D : D + 1].rearrange("p a b one -> p (a b) (one)"),
                )
                aoU = wpool.tile([128, 2, HEADS, D], F16, tag="aoU")
                nc.vector.tensor_tensor(
                    aoU[:], yo[:, :, :, 0:D], ivd[:].broadcast_to([128, 2, HEADS, D]), MUL
                )
                st["aoU"] = aoU

            def stage2b_b(st):
                """Transpose + project."""
                aoU = st["aoU"]
                aot_ps = ps_m1.tile([128, 2, 2, 128], F16, tag="m1", name="aot_ps")
                for ts in range(2):
                    nc.tensor.transpose(
                        aot_ps[:, ts, 0, :],
                        aoU[:, ts, 0:4, :].rearrange("p h d -> p (h d)"),
                        id_sb[:],
                    )
                    nc.tensor.transpose(
                        aot_ps[0:64, ts, 1, :],
                        aoU[:, ts, 4:6, :].rearrange("p h d -> p (h d)"),
                        id_sb[:],
                    )
                aot = wpool.tile([128, 2, 2, 128], F16, tag="aots")
                nc.vector.tensor_copy(aot[:], aot_ps[:])
                yps = ps_m1.tile([128, 2, C], FP, tag="m1")
                for ts in range(2):
                    nc.tensor.matmul(
                        yps[:, ts, :],
                        aot[:, ts, 0, :],
                        wp_sb[:, 0, :],
                        start=True,
                        stop=False,
                    )
                    nc.tensor.matmul(
                        yps[:, ts, :],
                        aot[0:64, ts, 1, :],
                        wp_sb[0:64, 1, :],
                        start=False,
                        stop=True,
                    )
                st["yps"] = yps

            def stage2b_c(w, st):
                """Descale + store."""
                ysb = wpool.tile([128, 2, C], F16, tag="ysb")
                nc.scalar.activation(ysb[:], st["yps"], COPY, scale=Y_DESCALE)
                nc.sync.dma_start(y_d[w], ysb[:])

            window = []  # (w, state) pending retirement, newest last
            for it in range(WPC):
                cur = stage1a(it)
                p_alloc(cur)
                old = window.pop(0) if len(window) >= 2 else None
                if old is not None:
                    stage2b_a(old[1])
                if window:
                    pv_alloc(window[-1][1])
                tail = it >= WPC - 2
                for ph in range(3):
                    if window:
                        pv_den_chunk(window[-1][1], 2 * ph)
                        pv_den_chunk(window[-1][1], 2 * ph + 1)
                    scores_phase(cur, ph)
                    for h in (2 * ph, 2 * ph + 1):
                        p_mult_group(
                            cur, h, h + 1,
                            nc.vector if (h < PSPLIT or tail) else nc.gpsimd,
                        )
                    if ph == 1:
                        if old is not None:
                            stage2b_b(old[1])
                        vaug_copy(cur, it)
                if old is not None:
                    stage2b_c(*old)
                window.append((it, cur))
            (w30, st30), (w31, st31) = window
            stage2b_a(st30)
            pv_alloc(st31)
            for h in range(HEADS):
                pv_den_chunk(st31, h)
            stage2b_b(st30)
            stage2b_a(st31)
            stage2b_c(w30, st30)
            stage2b_b(st31)
            stage2b_c(w31, st31)

    nc.compile()
    return nc


def _prep_inputs(inputs):
    x = np.asarray(inputs["x"], np.float32)
    mask = np.asarray(inputs["mask"], np.float32)
    w_qkv = np.asarray(inputs["w_qkv"], np.float32)
    b_qkv = np.asarray(inputs["b_qkv"], np.float32)
    w_proj = np.asarray(inputs["w_proj"], np.float32)
    b_proj = np.asarray(inputs["b_proj"], np.float32)
    H, W = int(inputs["H"]), int(inputs["W"])

    rpb = _pos_bias_host(
        H,
        W,
        *[
            np.asarray(inputs[k], np.float32)
            for k in (
                "pw0", "pb0", "g1", "be1", "w1", "b1",
                "g2", "be2", "w2", "b2", "g3", "be3", "w3", "b3",
            )
        ],
    )

    # EM[mb, h, m, n] = exp(mask[mb, n, m] + rpb[n, m, h]), device layout
    # [mb][p, h, mt, n] with m = 128*mt + p.
    bias = mask.transpose(0, 2, 1)[:, None] + rpb.transpose(2, 1, 0)[None]
    em = np.exp(bias).reshape(NMASK, HEADS, 2, 128, N).transpose(0, 3, 1, 2, 4)
    em = np.ascontiguousarray(em).astype(np.float16)  # [64, 128, 6, 2, 256]

    scale = float(D) ** -0.5
    wq = w_qkv[:, 0:C] * scale
    wk = w_qkv[:, C : 2 * C]
    wqk = np.zeros((C, 512), np.float32)
    wqk[:, 0:128] = wq[:, 0:128]
    wqk[:, 128:256] = wk[:, 0:128]
    wqk[:, 256:320] = wq[:, 128:C]
    wqk[:, 384:448] = wk[:, 128:C]
    wqk16 = np.zeros((2, 128, 512), np.float16)
    wqk16[0] = wqk[0:128]
    wqk16[1, 0:64] = wqk[128:C]
    wv16 = np.zeros((2, 128, C), np.float16)
    wv16[0] = w_qkv[0:128, 2 * C :]
    wv16[1, 0:64] = w_qkv[128:C, 2 * C :]
    wp16 = np.zeros((2, 128, C), np.float16)
    wp16[0] = w_proj[0:128]
    wp16[1, 0:64] = w_proj[128:C]
    ident = np.eye(128, dtype=np.float16)

    # per-core x^T chunks [w, 2, 128, 256]
    xt16 = np.zeros((B, 2, 128, N), np.float16)
    xtr = x.transpose(0, 2, 1)  # [B, C, N]
    xt16[:, 0] = xtr[:, 0:128]
    xt16[:, 1, 0:64] = xtr[:, 128:C]

    in_maps = []
    for core in range(NCORES):
        bs = [_win_to_b(core, w) for w in range(WPC)]
        in_maps.append(
            {
                "xt": np.ascontiguousarray(xt16[bs]),
                "em": np.ascontiguousarray(em[MPC * core : MPC * (core + 1)]),
                "wqk": wqk16,
                "wv": wv16,
                "wp": wp16,
                "ident": ident,
            }
        )
    # host-side constant correction for (generally zero) v/proj biases.
    ycorr = None
    if np.any(b_qkv != 0.0) or np.any(b_proj != 0.0):
        bv = b_qkv[2 * C :]
        ycorr = (bv @ w_proj + b_proj).astype(np.float32)
        if np.any(b_qkv[: 2 * C] != 0.0):
            raise NotImplementedError("nonzero q/k bias not supported by fast path")
    return in_maps, ycorr


def _assemble(results, ycorr):
    out = np.empty((B, N, C), np.float32)
    for core in range(NCORES):
        y = results[core]["y"]  # [WPC, 128, 2, C] fp16
        for w in range(WPC):
            b = _win_to_b(core, w)
            out[b] = y[w].transpose(1, 0, 2).reshape(N, C).astype(np.float32)
    if ycorr is not None:
        out += ycorr
    return out


def run(inputs, trace=False):
    from concourse.bass_utils import run_bass_kernel_spmd

    if "nc" not in _CACHE:
        _CACHE["nc"] = _build_nc()
    in_maps, ycorr = _prep_inputs(inputs)
    res = run_bass_kernel_spmd(
        _CACHE["nc"],
        in_maps,
        core_ids=list(range(NCORES)),
        trace=trace,
        trace_cores=[0] if trace else None,
    )
    return _assemble(res.results, ycorr), res


def get_nc():
    if "nc" not in _CACHE:
        _CACHE["nc"] = _build_nc()
    return _CACHE["nc"]


def kernel(**inputs):
    out, _ = run(inputs, trace=bool(int(os.environ.get("KERNEL_TRACE", "0"))))
    return out


# revision 31
# speedup vs baseline: 1.5672x; 1.0017x over previous
"""Trainium2 Bass kernel for nn_Attention_867583394433 (sparse window attention).

Strategy (8 NeuronCores, pure data parallel over windows B_=256 -> 32/core):
  - Host precomputes the tiny position-MLP -> relative-position-bias table and
    folds it with the additive mask into a multiplicative table
    EM[mask, head] = exp(rpb + mask) (fp16), laid out to match the on-chip
    transposed-score layout.  Windows are assigned so each core only touches
    8 distinct masks (mask index = b % 64) and EM stays SBUF-resident.
  - Device computes, per window, in transposed score layout S^T[m, n]
    (key index m on partitions, query index n on free dim):
      qk^T channel-layout matmuls -> scores (K=32, row-tiled)
      -> exp on ScalarE -> P = exp(S^T) * EM elementwise, split between
      VectorE and GpSimd (both only touch SBUF) to balance engines
      -> PV and denominator (ones-matmul, col-tiled into matching partition
         rows, packed [96, 2, 256]) on TensorE -> fast reciprocal + fused
         normalize on VectorE -> output projection -> fp16 DMA out.
  - The d^-0.5 attention scale is folded into wq on the host; the (zero)
    qkv/proj biases are folded into a host-side constant correction.
"""

import os

import numpy as np

HEADS = 6
D = 32
C = 192
N = 256
B = 256
NMASK = 64
POS_DIM = 12
EPS = 1e-5
NCORES = 8
WPC = B // NCORES  # 32 windows per core
MPC = NMASK // NCORES  # 8 masks per core
REP = B // NMASK  # 4 windows sharing one mask

S_AO = 16.0  # aoT = S_AO * attnout_true (via the ones value)
ONES_VAL = 1.0 / S_AO
Y_DESCALE = 1.0 / S_AO

# head split of the P = exp(S)*EM multiply: first PSPLIT heads on DVE,
# the rest on GpSimd.
PSPLIT = int(os.environ.get("PSPLIT", "3"))

_CACHE = {}


def _win_to_b(core, w):
    """Window order within a core: mask-major.  w = j*REP + k  ->  b."""
    j, k = divmod(w, REP)
    return NMASK * k + MPC * core + j


def _ln_np(x, g, b):
    m = x.mean(-1, keepdims=True)
    v = x.var(-1, keepdims=True)
    return (x - m) / np.sqrt(v + EPS) * g + b


def _pos_bias_host(H, W, pw0, pb0, g1, be1, w1, b1, g2, be2, w2, b2, g3, be3, w3, b3):
    """Replicates the reference position MLP + gather -> rpb [N, N, HEADS]."""
    H = int(H)
    W = int(W)
    ph = np.arange(1 - H, H)
    pw = np.arange(1 - W, W)
    biases = (
        np.stack(np.meshgrid(ph, pw, indexing="ij")).reshape(2, -1).T.astype(np.float32)
    )
    pos = biases @ pw0 + pb0
    pos = np.maximum(_ln_np(pos, g1, be1), 0.0) @ w1 + b1
    pos = np.maximum(_ln_np(pos, g2, be2), 0.0) @ w2 + b2
    pos = np.maximum(_ln_np(pos, g3, be3), 0.0) @ w3 + b3
    coords = np.stack(np.meshgrid(np.arange(H), np.arange(W), indexing="ij")).reshape(
        2, -1
    )
    rel = coords[:, :, None] - coords[:, None, :]
    rpi = (rel[0] + H - 1) * (2 * W - 1) + (rel[1] + W - 1)
    return pos[rpi]  # [N, N, HEADS] fp32


def _build_nc():
    import concourse.tile as tile
    from concourse import bacc, mybir

    FP = mybir.dt.float32
    F16 = mybir.dt.float16
    EXP = mybir.ActivationFunctionType.Exp
    COPY = mybir.ActivationFunctionType.Copy
    MUL = mybir.AluOpType.mult

    nc = bacc.Bacc("TRN2", target_bir_lowering=False, debug=False)
    # x^T feature chunks: [w, chunk(2), 128, 256]; chunk 1 holds features
    # 128:192 in rows 0:64 (rows 64:128 are zero padding).
    xt_d = nc.dram_tensor("xt", [WPC, 2, 128, N], F16, kind="ExternalInput")
    em_d = nc.dram_tensor("em", [MPC, 128, HEADS, 2, N], F16, kind="ExternalInput")
    wqk_d = nc.dram_tensor("wqk", [2, 128, 512], F16, kind="ExternalInput")
    wv_d = nc.dram_tensor("wv", [2, 128, C], F16, kind="ExternalInput")
    wp_d = nc.dram_tensor("wp", [2, 128, C], F16, kind="ExternalInput")
    id_d = nc.dram_tensor("ident", [128, 128], F16, kind="ExternalInput")
    y_d = nc.dram_tensor("y", [WPC, 128, 2, C], F16, kind="ExternalOutput")

    with tile.TileContext(nc) as tc:
        with (
            tc.tile_pool(name="const", bufs=1) as cpool,
            tc.tile_pool(name="win", bufs=int(os.environ.get("WBUFS", "3"))) as wpool,
            tc.tile_pool(name="big", bufs=int(os.environ.get("BBUFS", "4"))) as bpool,
            tc.tile_pool(name="pbig", bufs=int(os.environ.get("PBUFS", "3"))) as ppool,
            tc.tile_pool(name="ps_qk", bufs=int(os.environ.get("QKBUFS", "1")), space="PSUM") as ps_qk,
            tc.tile_pool(name="ps_sc", bufs=2, space="PSUM") as ps_sc,
            tc.tile_pool(name="ps_m1", bufs=2, space="PSUM") as ps_m1,
            tc.tile_pool(name="ps_yo", bufs=int(os.environ.get("YOBUFS", "1")), space="PSUM") as ps_yo,
        ):
            # ---- resident constants ----
            # (wv/wp/first-em DMAs are emitted from inside stage1a(0) so the
            # first window's x tile isn't queued behind them on the SP FIFO)
            em_sb = cpool.tile([128, MPC, HEADS, 2, N], F16)
            em_loaded = set()
            wqk_sb = cpool.tile([128, 2, 512], F16)
            nc.sync.dma_start(wqk_sb[:, 0], wqk_d[0])
            nc.sync.dma_start(wqk_sb[0:64, 1], wqk_d[1, 0:64])
            wv_sb = cpool.tile([128, 2, C], F16)
            wp_sb = cpool.tile([128, 2, C], F16)
            id_sb = cpool.tile([128, 128], F16)
            vaug_ring = cpool.tile([128, 2, 2, HEADS, D + 1], F16)
            nc.gpsimd.memset(vaug_ring[:, :, :, :, D : D + 1], ONES_VAL)

            # scores head -> (qk m-tile, partition row) maps
            q_loc = [(0, 32 * h) for h in range(4)] + [(2, 32 * (h - 4)) for h in (4, 5)]
            k_loc = [(1, 32 * h) for h in range(4)] + [(3, 32 * (h - 4)) for h in (4, 5)]

            def stage1a(w):
                """DMA x^T, qk matmuls + copy, v matmuls + copy, for window w."""
                j = w // REP
                xa = wpool.tile([128, 2, N], F16, tag="xa")
                nc.sync.dma_start(xa[:, 0, :], xt_d[w, 0])
                nc.sync.dma_start(xa[0:64, 1, :], xt_d[w, 1, 0:64])
                if w == 0:
                    nc.sync.dma_start(wv_sb[:, 0], wv_d[0])
                    nc.sync.dma_start(wv_sb[0:64, 1], wv_d[1, 0:64])
                for jl in {j, min((w + 2) // REP, MPC - 1)}:
                    if jl not in em_loaded:
                        em_loaded.add(jl)
                        nc.sync.dma_start(em_sb[:, jl], em_d[jl])
                if w == 0:
                    nc.sync.dma_start(wp_sb[:, 0], wp_d[0])
                    nc.sync.dma_start(wp_sb[0:64, 1], wp_d[1, 0:64])
                    nc.sync.dma_start(id_sb[:], id_d[:])

                qkT = wpool.tile([128, 4, N], F16, tag="qkT")
                qhalf = []
                for half in range(2):
                    qkps = ps_qk.tile(
                        [128, 2, N], FP, tag="qk", name="qkps"
                    )
                    qhalf.append(qkps)
                    for tt in range(2):
                        t = 2 * half + tt
                        nc.tensor.matmul(
                            qkps[:, tt, :],
                            wqk_sb[:, 0, 128 * t : 128 * (t + 1)],
                            xa[:, 0, :],
                            start=True,
                            stop=False,
                        )
                        nc.tensor.matmul(
                            qkps[:, tt, :],
                            wqk_sb[0:64, 1, 128 * t : 128 * (t + 1)],
                            xa[0:64, 1, :],
                            start=False,
                            stop=True,
                        )
                if int(os.environ.get("QKT_SPLIT", "0")):
                    nc.vector.tensor_copy(qkT[:, 0:2], qhalf[0][:])
                    nc.scalar.copy(qkT[:, 2:4], qhalf[1][:])
                else:
                    nc.vector.tensor_copy(qkT[:, 0:2], qhalf[0][:])
                    nc.vector.tensor_copy(qkT[:, 2:4], qhalf[1][:])

                vps = ps_m1.tile([128, 2, C], FP, tag="m1")
                for mt in range(2):
                    nc.tensor.matmul(
                        vps[:, mt, :],
                        xa[:, 0, 128 * mt : 128 * (mt + 1)],
                        wv_sb[:, 0, :],
                        start=True,
                        stop=False,
                    )
                    nc.tensor.matmul(
                        vps[:, mt, :],
                        xa[0:64, 1, 128 * mt : 128 * (mt + 1)],
                        wv_sb[0:64, 1, :],
                        start=False,
                        stop=True,
                    )
                es = bpool.tile([128, HEADS, 2, N], F16, tag="es")
                return {"j": j, "qkT": qkT, "vps": vps, "es": es}

            def vaug_copy(st, w):
                vaug = vaug_ring[:, w % 2]
                nc.vector.tensor_copy(
                    vaug[:, :, :, 0:D],
                    st["vps"][:].rearrange("p mt (h d) -> p mt h d", h=HEADS),
                )
                st["vaug"] = vaug

            def scores_phase(st, ph):
                """2 heads of S^T matmuls + one exp for this window."""
                qkT = st["qkT"]
                es = st["es"]
                scps = ps_sc.tile([128, 2, 2, N], FP, tag="sc", name="scps")
                for hh in range(2):
                    h = 2 * ph + hh
                    qt, qr = q_loc[h]
                    kt, kr = k_loc[h]
                    for mt in range(2):
                        nc.tensor.matmul(
                            scps[:, hh, mt, :],
                            qkT[kr : kr + 32, kt, 128 * mt : 128 * (mt + 1)],
                            qkT[qr : qr + 32, qt, :],
                            start=True,
                            stop=True,
                            tile_position=(kr, 0),
                        )
                nc.scalar.activation(es[:, 2 * ph : 2 * ph + 2], scps[:], EXP)

            def p_alloc(st):
                st["p"] = ppool.tile([128, HEADS, 2, N], F16, tag="P", name="p_t")

            def p_mult_group(st, h0, h1, eng):
                """P(w)[h0:h1] = exp(S^T) * EM as one elementwise op."""
                p_t = st["p"]
                j = st["j"]
                es = st["es"]
                eng.tensor_tensor(
                    p_t[:, h0:h1], es[:, h0:h1], em_sb[:, j, h0:h1], MUL
                )

            def pv_alloc(st):
                st["yo"] = ps_yo.tile([128, 2, HEADS, D + 1], FP, tag="yo", name="yo")

            def pv_den_chunk(st, h):
                """One head of transposed PV+den matmuls: P-block stationary,
                [v | ones] moving -> out [128 queries, 33] (ap_size 33)."""
                p_t = st["p"]
                vaug = st["vaug"]
                for ts in range(2):
                    for mt in range(2):
                        nc.tensor.matmul(
                            st["yo"][:, ts, h, :],
                            p_t[:, h, mt, 128 * ts : 128 * (ts + 1)],
                            vaug[:, mt, h, :],
                            start=(mt == 0),
                            stop=(mt == 1),
                        )

            def stage2b_a(st):
                """Normalize (broadcast divide) on VectorE -- emitted early."""
                yo = st["yo"]
                ivd = wpool.tile([128, 2, HEADS, 1], FP, tag="ivd")
                nc.vector.reciprocal_approx_fast(
                    ivd[:].rearrange("p a b one -> p (a b) (one)"),
                    yo[:, :, :, D : D + 1].rearrange("p a b one -> p (a b) (one)"),
                )
                aoU = wpool.tile([128, 2, HEADS, D], F16, tag="aoU")
                nc.vector.tensor_tensor(
                    aoU[:], yo[:, :, :, 0:D], ivd[:].broadcast_to([128, 2, HEADS, D]), MUL
                )
                st["aoU"] = aoU

            def stage2b_b(st):
                """Transpose + project."""
                aoU = st["aoU"]
                aot_ps = ps_m1.tile([128, 2, 2, 128], F16, tag="m1", name="aot_ps")
                for ts in range(2):
                    nc.tensor.transpose(
                        aot_ps[:, ts, 0, :],
                        aoU[:, ts, 0:4, :].rearrange("p h d -> p (h d)"),
                        id_sb[:],
                    )
                    nc.tensor.transpose(
                        aot_ps[0:64, ts, 1, :],
                        aoU[:, ts, 4:6, :].rearrange("p h d -> p (h d)"),
                        id_sb[:],
                    )
                aot = wpool.tile([128, 2, 2, 128], F16, tag="aots")
                nc.vector.tensor_copy(aot[:], aot_ps[:])
                yps = ps_m1.tile([128, 2, C], FP, tag="m1")
                for ts in range(2):
                    nc.tensor.matmul(
                        yps[:, ts, :],
                        aot[:, ts, 0, :],
                        wp_sb[:, 0, :],
                        start=True,
                        stop=False,
                    )
                    nc.tensor.matmul(
                        yps[:, ts, :],
                        aot[0:64, ts, 1, :],
                        wp_sb[0:64, 1, :],
                        start=False,
                        stop=True,
                    )
                st["yps"] = yps

            def stage2b_c(w, st):
                """Descale + store."""
                ysb = wpool.tile([128, 2, C], F16, tag="ysb")
                nc.scalar.activation(ysb[:], st["yps"], COPY, scale=Y_DESCALE)
                nc.sync.dma_start(y_d[w], ysb[:])

            window = []  # (w, state) pending retirement, newest last
            for it in range(WPC):
                cur = stage1a(it)
                p_alloc(cur)
                old = window.pop(0) if len(window) >= 2 else None
                if old is not None:
                    stage2b_a(old[1])
                if window:
                    pv_alloc(window[-1][1])
                tail = it >= WPC - 2
                for ph in range(3):
                    if window:
                        pv_den_chunk(window[-1][1], 2 * ph)
                        pv_den_chunk(window[-1][1], 2 * ph + 1)
                    scores_phase(cur, ph)
                    for h in (2 * ph, 2 * ph + 1):
                        p_mult_group(
                            cur, h, h + 1,
                            nc.vector if (h < PSPLIT or tail) else nc.gpsimd,
                        )
                    if ph == 1:
                        if old is not None:
                            stage2b_b(old[1])
                        vaug_copy(cur, it)
                if old is not None:
                    stage2b_c(*old)
                window.append((it, cur))
            (w30, st30), (w31, st31) = window
            stage2b_a(st30)
            pv_alloc(st31)
            for h in range(HEADS):
                pv_den_chunk(st31, h)
            stage2b_b(st30)
            stage2b_a(st31)
            stage2b_c(w30, st30)
            stage2b_b(st31)
            stage2b_c(w31, st31)

    nc.compile()
    return nc


def _prep_inputs(inputs):
    x = np.asarray(inputs["x"], np.float32)
    mask = np.asarray(inputs["mask"], np.float32)
    w_qkv = np.asarray(inputs["w_qkv"], np.float32)
    b_qkv = np.asarray(inputs["b_qkv"], np.float32)
    w_proj = np.asarray(inputs["w_proj"], np.float32)
    b_proj = np.asarray(inputs["b_proj"], np.float32)
    H, W = int(inputs["H"]), int(inputs["W"])

    rpb = _pos_bias_host(
        H,
        W,
        *[
            np.asarray(inputs[k], np.float32)
            for k in (
                "pw0", "pb0", "g1", "be1", "w1", "b1",
                "g2", "be2", "w2", "b2", "g3", "be3", "w3", "b3",
            )
        ],
    )

    # EM[mb, h, m, n] = exp(mask[mb, n, m] + rpb[n, m, h]), device layout
    # [mb][p, h, mt, n] with m = 128*mt + p.
    bias = mask.transpose(0, 2, 1)[:, None] + rpb.transpose(2, 1, 0)[None]
    em = np.exp(bias).reshape(NMASK, HEADS, 2, 128, N).transpose(0, 3, 1, 2, 4)
    em = np.ascontiguousarray(em).astype(np.float16)  # [64, 128, 6, 2, 256]

    scale = float(D) ** -0.5
    wq = w_qkv[:, 0:C] * scale
    wk = w_qkv[:, C : 2 * C]
    wqk = np.zeros((C, 512), np.float32)
    wqk[:, 0:128] = wq[:, 0:128]
    wqk[:, 128:256] = wk[:, 0:128]
    wqk[:, 256:320] = wq[:, 128:C]
    wqk[:, 384:448] = wk[:, 128:C]
    wqk16 = np.zeros((2, 128, 512), np.float16)
    wqk16[0] = wqk[0:128]
    wqk16[1, 0:64] = wqk[128:C]
    wv16 = np.zeros((2, 128, C), np.float16)
    wv16[0] = w_qkv[0:128, 2 * C :]
    wv16[1, 0:64] = w_qkv[128:C, 2 * C :]
    wp16 = np.zeros((2, 128, C), np.float16)
    wp16[0] = w_proj[0:128]
    wp16[1, 0:64] = w_proj[128:C]
    ident = np.eye(128, dtype=np.float16)

    # per-core x^T chunks [w, 2, 128, 256]
    xt16 = np.zeros((B, 2, 128, N), np.float16)
    xtr = x.transpose(0, 2, 1)  # [B, C, N]
    xt16[:, 0] = xtr[:, 0:128]
    xt16[:, 1, 0:64] = xtr[:, 128:C]

    in_maps = []
    for core in range(NCORES):
        bs = [_win_to_b(core, w) for w in range(WPC)]
        in_maps.append(
            {
                "xt": np.ascontiguousarray(xt16[bs]),
                "em": np.ascontiguousarray(em[MPC * core : MPC * (core + 1)]),
                "wqk": wqk16,
                "wv": wv16,
                "wp": wp16,
                "ident": ident,
            }
        )
    # host-side constant correction for (generally zero) v/proj biases.
    ycorr = None
    if np.any(b_qkv != 0.0) or np.any(b_proj != 0.0):
        bv = b_qkv[2 * C :]
        ycorr = (bv @ w_proj + b_proj).astype(np.float32)
        if np.any(b_qkv[: 2 * C] != 0.0):
            raise NotImplementedError("nonzero q/k bias not supported by fast path")
    return in_maps, ycorr


def _assemble(results, ycorr):
    out = np.empty((B, N, C), np.float32)
    for core in range(NCORES):
        y = results[core]["y"]  # [WPC, 128, 2, C] fp16
        for w in range(WPC):
            b = _win_to_b(core, w)
            out[b] = y[w].transpose(1, 0, 2).reshape(N, C).astype(np.float32)
    if ycorr is not None:
        out += ycorr
    return out


def run(inputs, trace=False):
    from concourse.bass_utils import run_bass_kernel_spmd

    if "nc" not in _CACHE:
        _CACHE["nc"] = _build_nc()
    in_maps, ycorr = _prep_inputs(inputs)
    res = run_bass_kernel_spmd(
        _CACHE["nc"],
        in_maps,
        core_ids=list(range(NCORES)),
        trace=trace,
        trace_cores=[0] if trace else None,
    )
    return _assemble(res.results, ycorr), res


def get_nc():
    if "nc" not in _CACHE:
        _CACHE["nc"] = _build_nc()
    return _CACHE["nc"]


def kernel(**inputs):
    out, _ = run(inputs, trace=bool(int(os.environ.get("KERNEL_TRACE", "0"))))
    return out


# revision 34
# speedup vs baseline: 1.6070x; 1.0254x over previous
"""Trainium2 Bass kernel for nn_Attention_867583394433 (sparse window attention).

Strategy (8 NeuronCores, pure data parallel over windows B_=256 -> 32/core):
  - Host precomputes the tiny position-MLP -> relative-position-bias table and
    folds it with the additive mask into a multiplicative table
    EM[mask, head] = exp(rpb + mask) (fp16), laid out to match the on-chip
    transposed-score layout.  Windows are assigned so each core only touches
    8 distinct masks (mask index = b % 64) and EM stays SBUF-resident.
  - Device computes, per window, in transposed score layout S^T[m, n]
    (key index m on partitions, query index n on free dim):
      qk^T channel-layout matmuls -> scores (K=32, row-tiled)
      -> exp on ScalarE -> P = exp(S^T) * EM elementwise, split between
      VectorE and GpSimd (both only touch SBUF) to balance engines
      -> PV and denominator (ones-matmul, col-tiled into matching partition
         rows, packed [96, 2, 256]) on TensorE -> fast reciprocal + fused
         normalize on VectorE -> output projection -> fp16 DMA out.
  - The d^-0.5 attention scale is folded into wq on the host; the (zero)
    qkv/proj biases are folded into a host-side constant correction.
"""

import os

import numpy as np

HEADS = 6
D = 32
C = 192
N = 256
B = 256
NMASK = 64
POS_DIM = 12
EPS = 1e-5
NCORES = 8
WPC = B // NCORES  # 32 windows per core
MPC = NMASK // NCORES  # 8 masks per core
REP = B // NMASK  # 4 windows sharing one mask

S_AO = 16.0  # aoT = S_AO * attnout_true (via the ones value)
ONES_VAL = 1.0 / S_AO
Y_DESCALE = 1.0 / S_AO

# head split of the P = exp(S)*EM multiply: first PSPLIT heads on DVE,
# the rest on GpSimd.
PSPLIT = int(os.environ.get("PSPLIT", "3"))

_CACHE = {}


def _win_to_b(core, w):
    """Window order within a core: mask-major.  w = j*REP + k  ->  b."""
    j, k = divmod(w, REP)
    return NMASK * k + MPC * core + j


def _ln_np(x, g, b):
    m = x.mean(-1, keepdims=True)
    v = x.var(-1, keepdims=True)
    return (x - m) / np.sqrt(v + EPS) * g + b


def _pos_bias_host(H, W, pw0, pb0, g1, be1, w1, b1, g2, be2, w2, b2, g3, be3, w3, b3):
    """Replicates the reference position MLP + gather -> rpb [N, N, HEADS]."""
    H = int(H)
    W = int(W)
    ph = np.arange(1 - H, H)
    pw = np.arange(1 - W, W)
    biases = (
        np.stack(np.meshgrid(ph, pw, indexing="ij")).reshape(2, -1).T.astype(np.float32)
    )
    pos = biases @ pw0 + pb0
    pos = np.maximum(_ln_np(pos, g1, be1), 0.0) @ w1 + b1
    pos = np.maximum(_ln_np(pos, g2, be2), 0.0) @ w2 + b2
    pos = np.maximum(_ln_np(pos, g3, be3), 0.0) @ w3 + b3
    coords = np.stack(np.meshgrid(np.arange(H), np.arange(W), indexing="ij")).reshape(
        2, -1
    )
    rel = coords[:, :, None] - coords[:, None, :]
    rpi = (rel[0] + H - 1) * (2 * W - 1) + (rel[1] + W - 1)
    return pos[rpi]  # [N, N, HEADS] fp32


def _build_nc():
    import concourse.tile as tile
    from concourse import bacc, mybir

    FP = mybir.dt.float32
    F32R = mybir.dt.float32r
    F16 = mybir.dt.float16
    EXP = mybir.ActivationFunctionType.Exp
    COPY = mybir.ActivationFunctionType.Copy
    MUL = mybir.AluOpType.mult

    nc = bacc.Bacc("TRN2", target_bir_lowering=False, debug=False)
    # x^T feature chunks: [w, chunk(2), 128, 256]; chunk 1 holds features
    # 128:192 in rows 0:64 (rows 64:128 are zero padding).
    xt_d = nc.dram_tensor("xt", [WPC, 2, 128, N], F16, kind="ExternalInput")
    em_d = nc.dram_tensor("em", [MPC, 128, HEADS, 2, N], F16, kind="ExternalInput")
    wqk_d = nc.dram_tensor("wqk", [2, 128, 512], F16, kind="ExternalInput")
    wv_d = nc.dram_tensor("wv", [2, 128, C], F16, kind="ExternalInput")
    wp_d = nc.dram_tensor("wp", [2, 128, C], F16, kind="ExternalInput")
    id_d = nc.dram_tensor("ident", [128, 128], F16, kind="ExternalInput")
    y_d = nc.dram_tensor("y", [WPC, 128, 2, C], F16, kind="ExternalOutput")

    with tile.TileContext(nc) as tc:
        with (
            tc.tile_pool(name="const", bufs=1) as cpool,
            tc.tile_pool(name="win", bufs=int(os.environ.get("WBUFS", "3"))) as wpool,
            tc.tile_pool(name="big", bufs=int(os.environ.get("BBUFS", "4"))) as bpool,
            tc.tile_pool(name="pbig", bufs=int(os.environ.get("PBUFS", "3"))) as ppool,
            tc.tile_pool(name="ps_qk", bufs=int(os.environ.get("QKBUFS", "2")), space="PSUM") as ps_qk,
            tc.tile_pool(name="ps_sc", bufs=2, space="PSUM") as ps_sc,
            tc.tile_pool(name="ps_m1", bufs=int(os.environ.get("M1BUFS", "1")), space="PSUM") as ps_m1,
            tc.tile_pool(name="ps_yo", bufs=int(os.environ.get("YOBUFS", "1")), space="PSUM") as ps_yo,
        ):
            # ---- resident constants ----
            # (wv/wp/first-em DMAs are emitted from inside stage1a(0) so the
            # first window's x tile isn't queued behind them on the SP FIFO)
            em_sb = cpool.tile([128, MPC, HEADS, 2, N], F16)
            em_loaded = set()
            wqk_sb = cpool.tile([128, 2, 512], F16)
            nc.sync.dma_start(wqk_sb[:, 0], wqk_d[0])
            nc.sync.dma_start(wqk_sb[0:64, 1], wqk_d[1, 0:64])
            wv_sb = cpool.tile([128, 2, C], F16)
            wp_sb = cpool.tile([128, 2, C], F16)
            id_sb = cpool.tile([128, 128], F16)
            vaug_ring = cpool.tile([128, 2, 2, HEADS, D + 1], F16)
            nc.gpsimd.memset(vaug_ring[:, :, :, :, D : D + 1], ONES_VAL)

            # scores head -> (qk m-tile, partition row) maps
            q_loc = [(0, 32 * h) for h in range(4)] + [(2, 32 * (h - 4)) for h in (4, 5)]
            k_loc = [(1, 32 * h) for h in range(4)] + [(3, 32 * (h - 4)) for h in (4, 5)]

            def stage1a(w):
                """DMA x^T, qk matmuls + copy, v matmuls + copy, for window w."""
                j = w // REP
                xa = wpool.tile([128, 2, N], F16, tag="xa")
                nc.sync.dma_start(xa[:, 0, :], xt_d[w, 0])
                nc.sync.dma_start(xa[0:64, 1, :], xt_d[w, 1, 0:64])
                if w == 0:
                    nc.sync.dma_start(wv_sb[:, 0], wv_d[0])
                    nc.sync.dma_start(wv_sb[0:64, 1], wv_d[1, 0:64])
                for jl in {j, min((w + 2) // REP, MPC - 1)}:
                    if jl not in em_loaded:
                        em_loaded.add(jl)
                        nc.sync.dma_start(em_sb[:, jl], em_d[jl])
                if w == 0:
                    nc.sync.dma_start(wp_sb[:, 0], wp_d[0])
                    nc.sync.dma_start(wp_sb[0:64, 1], wp_d[1, 0:64])
                    nc.sync.dma_start(id_sb[:], id_d[:])

                qkT = wpool.tile([128, 4, N], F32R, tag="qkT")
                for half in range(2):
                    qkps = ps_qk.tile([128, 2, N], FP, tag="qk", name="qkps")
                    for tt in range(2):
                        t = 2 * half + tt
                        nc.tensor.matmul(
                            qkps[:, tt, :],
                            wqk_sb[:, 0, 128 * t : 128 * (t + 1)],
                            xa[:, 0, :],
                            start=True,
                            stop=False,
                        )
                        nc.tensor.matmul(
                            qkps[:, tt, :],
                            wqk_sb[0:64, 1, 128 * t : 128 * (t + 1)],
                            xa[0:64, 1, :],
                            start=False,
                            stop=True,
                        )
                    nc.vector.tensor_copy(qkT[:, 2 * half : 2 * half + 2], qkps[:])

                vps = ps_m1.tile([128, 2, C], FP, tag="m1")
                for mt in range(2):
                    nc.tensor.matmul(
                        vps[:, mt, :],
                        xa[:, 0, 128 * mt : 128 * (mt + 1)],
                        wv_sb[:, 0, :],
                        start=True,
                        stop=False,
                    )
                    nc.tensor.matmul(
                        vps[:, mt, :],
                        xa[0:64, 1, 128 * mt : 128 * (mt + 1)],
                        wv_sb[0:64, 1, :],
                        start=False,
                        stop=True,
                    )
                es = bpool.tile([128, HEADS, 2, N], F16, tag="es")
                return {"j": j, "qkT": qkT, "vps": vps, "es": es}

            def vaug_copy(st, w):
                vaug = vaug_ring[:, w % 2]
                nc.vector.tensor_copy(
                    vaug[:, :, :, 0:D],
                    st["vps"][:].rearrange("p mt (h d) -> p mt h d", h=HEADS),
                )
                st["vaug"] = vaug

            def scores_phase(st, ph):
                """2 heads of S^T matmuls + one exp for this window."""
                qkT = st["qkT"]
                es = st["es"]
                scps = ps_sc.tile([128, 2, 2, N], FP, tag="sc", name="scps")
                for hh in range(2):
                    h = 2 * ph + hh
                    qt, qr = q_loc[h]
                    kt, kr = k_loc[h]
                    for mt in range(2):
                        nc.tensor.matmul(
                            scps[:, hh, mt, :],
                            qkT[kr : kr + 32, kt, 128 * mt : 128 * (mt + 1)],
                            qkT[qr : qr + 32, qt, :],
                            start=True,
                            stop=True,
                            tile_position=(kr, 0),
                        )
                nc.scalar.activation(es[:, 2 * ph : 2 * ph + 2], scps[:], EXP)

            def p_alloc(st):
                st["p"] = ppool.tile([128, HEADS, 2, N], F16, tag="P", name="p_t")

            def p_mult_group(st, h0, h1, eng):
                """P(w)[h0:h1] = exp(S^T) * EM as one elementwise op."""
                p_t = st["p"]
                j = st["j"]
                es = st["es"]
                eng.tensor_tensor(
                    p_t[:, h0:h1], es[:, h0:h1], em_sb[:, j, h0:h1], MUL
                )

            def pv_alloc(st):
                st["yo"] = ps_yo.tile([128, 2, HEADS, D + 1], FP, tag="yo", name="yo")

            def pv_den_chunk(st, h):
                """One head of transposed PV+den matmuls: P-block stationary,
                [v | ones] moving -> out [128 queries, 33] (ap_size 33)."""
                p_t = st["p"]
                vaug = st["vaug"]
                for ts in range(2):
                    for mt in range(2):
                        nc.tensor.matmul(
                            st["yo"][:, ts, h, :],
                            p_t[:, h, mt, 128 * ts : 128 * (ts + 1)],
                            vaug[:, mt, h, :],
                            start=(mt == 0),
                            stop=(mt == 1),
                        )

            def stage2b_a(st):
                """Normalize (broadcast divide) on VectorE -- emitted early."""
                yo = st["yo"]
                ivd = wpool.tile([128, 2, HEADS, 1], FP, tag="ivd")
                nc.vector.reciprocal_approx_fast(
                    ivd[:].rearrange("p a b one -> p (a b) (one)"),
                    yo[:, :, :, D : D + 1].rearrange("p a b one -> p (a b) (one)"),
                )
                aoU = wpool.tile([128, 2, HEADS, D], F16, tag="aoU")
                nc.vector.tensor_tensor(
                    aoU[:], yo[:, :, :, 0:D], ivd[:].broadcast_to([128, 2, HEADS, D]), MUL
                )
                st["aoU"] = aoU

            def stage2b_b(st):
                """Transpose + project."""
                aoU = st["aoU"]
                aot_ps = ps_m1.tile([128, 2, 2, 128], F16, tag="m1", name="aot_ps")
                for ts in range(2):
                    nc.tensor.transpose(
                        aot_ps[:, ts, 0, :],
                        aoU[:, ts, 0:4, :].rearrange("p h d -> p (h d)"),
                        id_sb[:],
                    )
                    nc.tensor.transpose(
                        aot_ps[0:64, ts, 1, :],
                        aoU[:, ts, 4:6, :].rearrange("p h d -> p (h d)"),
                        id_sb[:],
                    )
                aot = wpool.tile([128, 2, 2, 128], F16, tag="aots")
                nc.vector.tensor_copy(aot[:], aot_ps[:])
                yps = ps_m1.tile([128, 2, C], FP, tag="m1")
                for ts in range(2):
                    nc.tensor.matmul(
                        yps[:, ts, :],
                        aot[:, ts, 0, :],
                        wp_sb[:, 0, :],
                        start=True,
                        stop=False,
                    )
                    nc.tensor.matmul(
                        yps[:, ts, :],
                        aot[0:64, ts, 1, :],
                        wp_sb[0:64, 1, :],
                        start=False,
                        stop=True,
                    )
                st["yps"] = yps

            def stage2b_c(w, st):
                """Descale + store."""
                ysb = wpool.tile([128, 2, C], F16, tag="ysb")
                nc.scalar.activation(ysb[:], st["yps"], COPY, scale=Y_DESCALE)
                nc.sync.dma_start(y_d[w], ysb[:])

            window = []  # (w, state) pending retirement, newest last
            for it in range(WPC):
                cur = stage1a(it)
                p_alloc(cur)
                old = window.pop(0) if len(window) >= 2 else None
                if old is not None:
                    stage2b_a(old[1])
                if window:
                    pv_alloc(window[-1][1])
                tail = it >= WPC - 2
                for ph in range(3):
                    if window:
                        pv_den_chunk(window[-1][1], 2 * ph)
                        pv_den_chunk(window[-1][1], 2 * ph + 1)
                    scores_phase(cur, ph)
                    for h in (2 * ph, 2 * ph + 1):
                        p_mult_group(
                            cur, h, h + 1,
                            nc.vector if (h < PSPLIT or tail) else nc.gpsimd,
                        )
                    if ph == 1:
                        if old is not None:
                            stage2b_b(old[1])
                        vaug_copy(cur, it)
                if old is not None:
                    stage2b_c(*old)
                window.append((it, cur))
            (w30, st30), (w31, st31) = window
            stage2b_a(st30)
            pv_alloc(st31)
            for h in range(HEADS):
                pv_den_chunk(st31, h)
            stage2b_b(st30)
            stage2b_a(st31)
            stage2b_c(w30, st30)
            stage2b_b(st31)
            stage2b_c(w31, st31)

    nc.compile()
    return nc


def _prep_inputs(inputs):
    x = np.asarray(inputs["x"], np.float32)
    mask = np.asarray(inputs["mask"], np.float32)
    w_qkv = np.asarray(inputs["w_qkv"], np.float32)
    b_qkv = np.asarray(inputs["b_qkv"], np.float32)
    w_proj = np.asarray(inputs["w_proj"], np.float32)
    b_proj = np.asarray(inputs["b_proj"], np.float32)
    H, W = int(inputs["H"]), int(inputs["W"])

    rpb = _pos_bias_host(
        H,
        W,
        *[
            np.asarray(inputs[k], np.float32)
            for k in (
                "pw0", "pb0", "g1", "be1", "w1", "b1",
                "g2", "be2", "w2", "b2", "g3", "be3", "w3", "b3",
            )
        ],
    )

    # EM[mb, h, m, n] = exp(mask[mb, n, m] + rpb[n, m, h]), device layout
    # [mb][p, h, mt, n] with m = 128*mt + p.
    bias = mask.transpose(0, 2, 1)[:, None] + rpb.transpose(2, 1, 0)[None]
    em = np.exp(bias).reshape(NMASK, HEADS, 2, 128, N).transpose(0, 3, 1, 2, 4)
    em = np.ascontiguousarray(em).astype(np.float16)  # [64, 128, 6, 2, 256]

    scale = float(D) ** -0.5
    wq = w_qkv[:, 0:C] * scale
    wk = w_qkv[:, C : 2 * C]
    wqk = np.zeros((C, 512), np.float32)
    wqk[:, 0:128] = wq[:, 0:128]
    wqk[:, 128:256] = wk[:, 0:128]
    wqk[:, 256:320] = wq[:, 128:C]
    wqk[:, 384:448] = wk[:, 128:C]
    wqk16 = np.zeros((2, 128, 512), np.float16)
    wqk16[0] = wqk[0:128]
    wqk16[1, 0:64] = wqk[128:C]
    wv16 = np.zeros((2, 128, C), np.float16)
    wv16[0] = w_qkv[0:128, 2 * C :]
    wv16[1, 0:64] = w_qkv[128:C, 2 * C :]
    wp16 = np.zeros((2, 128, C), np.float16)
    wp16[0] = w_proj[0:128]
    wp16[1, 0:64] = w_proj[128:C]
    ident = np.eye(128, dtype=np.float16)

    # per-core x^T chunks [w, 2, 128, 256]
    xt16 = np.zeros((B, 2, 128, N), np.float16)
    xtr = x.transpose(0, 2, 1)  # [B, C, N]
    xt16[:, 0] = xtr[:, 0:128]
    xt16[:, 1, 0:64] = xtr[:, 128:C]

    in_maps = []
    for core in range(NCORES):
        bs = [_win_to_b(core, w) for w in range(WPC)]
        in_maps.append(
            {
                "xt": np.ascontiguousarray(xt16[bs]),
                "em": np.ascontiguousarray(em[MPC * core : MPC * (core + 1)]),
                "wqk": wqk16,
                "wv": wv16,
                "wp": wp16,
                "ident": ident,
            }
        )
    # host-side constant correction for (generally zero) v/proj biases.
    ycorr = None
    if np.any(b_qkv != 0.0) or np.any(b_proj != 0.0):
        bv = b_qkv[2 * C :]
        ycorr = (bv @ w_proj + b_proj).astype(np.float32)
        if np.any(b_qkv[: 2 * C] != 0.0):
            raise NotImplementedError("nonzero q/k bias not supported by fast path")
    return in_maps, ycorr


def _assemble(results, ycorr):
    out = np.empty((B, N, C), np.float32)
    for core in range(NCORES):
        y = results[core]["y"]  # [WPC, 128, 2, C] fp16
        for w in range(WPC):
            b = _win_to_b(core, w)
            out[b] = y[w].transpose(1, 0, 2).reshape(N, C).astype(np.float32)
    if ycorr is not None:
        out += ycorr
    return out


def run(inputs, trace=False):
    from concourse.bass_utils import run_bass_kernel_spmd

    if "nc" not in _CACHE:
        _CACHE["nc"] = _build_nc()
    in_maps, ycorr = _prep_inputs(inputs)
    res = run_bass_kernel_spmd(
        _CACHE["nc"],
        in_maps,
        core_ids=list(range(NCORES)),
        trace=trace,
        trace_cores=[0] if trace else None,
    )
    return _assemble(res.results, ycorr), res


def get_nc():
    if "nc" not in _CACHE:
        _CACHE["nc"] = _build_nc()
    return _CACHE["nc"]


def kernel(**inputs):
    out, _ = run(inputs, trace=bool(int(os.environ.get("KERNEL_TRACE", "0"))))
    return out
